# revision 15
# baseline (speedup 1.0000x reference)
"""Trainium2 Bass kernel for nn_AttentionBlock (GroupNorm + 8-head self-attention + proj + residual).

Full inputs in, full output out. Sharding: 8 cores = 2 batches x 4-way split of
the 4096 query pixels. Each core runs an identical SPMD program on per-core
input data (x rolled so its 1024 query pixels sit first; attention and
groupnorm are permutation-invariant over keys/pixels, so rotation is exact).

Host-side folding (exact, fp32): groupnorm h = a(.)x + b folds into the QKV
weights: ws_q = diag(a) Wq^T (fp8) with qbias = Wq b + wq_b; k's constant
cancels in softmax; v's constant rides through (cout = proj_w (Wv b + wv_b) +
proj_b added at the end). The residual+cout constant is pre-added on the host
into the rsdc tensor.

Device-side:
  QKV GEMMs in fp8 DoubleRow, pidx-outer so each stationary weight load
  covers several matmuls.  Attention: S^T tiles (keys on partitions); QK is
  emitted as four 64x64-quadrant matmuls (explicit tile_position) so all four
  run concurrently in the PE array; PV and the ones (denominator) matmuls are
  column-tiled pairs.  exp is round-robined over ScalarE (exact, 1/8 scale
  fused), Pool/GpSimd and DVE (int16-Schraudolph); o_norm = oA * recip(oB);
  y^T = proj_w o_norm^T + rsdc, DMA'd out per 512-chunk.
"""

import numpy as np
import ml_dtypes
from contextlib import ExitStack

import concourse.bacc as bacc
import concourse.tile as tile
import concourse.mybir as mybir
from concourse.bass_utils import run_bass_kernel_spmd

BF16 = ml_dtypes.bfloat16
FP8 = ml_dtypes.float8_e4m3
F32 = np.float32

P = 128          # partitions
C = 512          # channels
NH = 8
HS = 64
N = 4096         # pixels (keys)
NQ = 1024        # queries per core
CT = 4           # channel tiles of 128
MT = 32          # m (key) tiles of 128
EPS = 1e-5
SCH_A16 = float(2.0 ** 7 / np.log(2.0))   # int16 Schraudolph exp
SCH_B16 = 16250.4062

dt = mybir.dt
AOT = mybir.AluOpType
ACTF = mybir.ActivationFunctionType
AXT = mybir.AxisListType

_CACHE = {}

# exp engine schedule per score tile: P=pool/gpsimd, A=scalar-exact, D=dve
EXPSCHED = "AD"


def build_program():
    nc = bacc.Bacc("TRN2", target_bir_lowering=False, debug=False, num_devices=8)

    xb_d = nc.dram_tensor("xb", [C, N], dt.float8e4, kind="ExternalInput")
    rsdc_d = nc.dram_tensor("rsdc", [C, NQ], dt.float32, kind="ExternalInput")
    wsq_d = nc.dram_tensor("wsq", [C, C], dt.float8e4, kind="ExternalInput")
    wsk_d = nc.dram_tensor("wsk", [C, C], dt.float8e4, kind="ExternalInput")
    wsv_d = nc.dram_tensor("wsv", [C, C], dt.float8e4, kind="ExternalInput")
    wp_d = nc.dram_tensor("wpT", [C, C], dt.float8e4, kind="ExternalInput")
    qb_d = nc.dram_tensor("qb4", [P, CT], dt.float32, kind="ExternalInput")
    y_d = nc.dram_tensor("y", [C, NQ], dt.float32, kind="ExternalOutput")

    with tile.TileContext(nc) as tc, ExitStack() as ctx:
        const = ctx.enter_context(tc.tile_pool(name="const", bufs=1))
        wpool = ctx.enter_context(tc.tile_pool(name="wpool", bufs=1))
        xpool = ctx.enter_context(tc.tile_pool(name="xpool", bufs=1))
        kpool = ctx.enter_context(tc.tile_pool(name="kpool", bufs=1))
        qpool = ctx.enter_context(tc.tile_pool(name="qpool", bufs=1))
        vpool = ctx.enter_context(tc.tile_pool(name="vpool", bufs=1))
        epool = ctx.enter_context(tc.tile_pool(name="epool", bufs=3))
        onpool = ctx.enter_context(tc.tile_pool(name="onpool", bufs=1))
        rpool = ctx.enter_context(tc.tile_pool(name="rpool", bufs=2))
        rsdpool = ctx.enter_context(tc.tile_pool(name="rsdpool", bufs=2))
        outpool = ctx.enter_context(tc.tile_pool(name="outpool", bufs=2))

        # ---------------- input DMA ----------------
        # tiny fp8 q/k weights first (they gate the first GEMMs), then the
        # 2MB xb split over four queues; wsv/qb next (gate V/q-bias); wp and
        # the residual go last (needed only at the tail).
        ws = {}
        w = wpool.tile([P, CT * C], dt.float8e4, tag="ws_q", name="ws_q")
        for kt in range(CT):
            nc.sync.dma_start(
                w[:, kt * C:(kt + 1) * C], wsq_d.ap()[kt * P:(kt + 1) * P, :]
            )
        ws["q"] = w

        xpair = [xpool.tile([P, 2 * N], dt.float8e4, name=f"xp{pidx}")
                 for pidx in range(2)]
        for t, eng in ((0, nc.sync), (1, nc.scalar), (2, nc.gpsimd), (3, nc.gpsimd)):
            eng.dma_start(
                xpair[t // 2][:, (t % 2) * N:(t % 2 + 1) * N],
                xb_d.ap()[t * P:(t + 1) * P, :],
            )

        w = wpool.tile([P, CT * C], dt.float8e4, tag="ws_k", name="ws_k")
        for kt in range(CT):
            nc.scalar.dma_start(
                w[:, kt * C:(kt + 1) * C], wsk_d.ap()[kt * P:(kt + 1) * P, :]
            )
        ws["k"] = w
        qb4 = const.tile([P, CT], dt.float32)
        nc.sync.dma_start(qb4[:], qb_d.ap())
        wsv = wpool.tile([P, CT * C], dt.float8e4, tag="ws_v", name="ws_v")
        for kt in range(CT):
            eng = nc.scalar if kt % 2 == 0 else nc.sync
            eng.dma_start(
                wsv[:, kt * C:(kt + 1) * C], wsv_d.ap()[kt * P:(kt + 1) * P, :]
            )
        ws["v"] = wsv
        wp = wpool.tile([P, CT * C], dt.float8e4, tag="w_p", name="w_p")
        for kt in range(CT):
            eng = nc.gpsimd if kt % 2 == 0 else nc.sync
            eng.dma_start(
                wp[:, kt * C:(kt + 1) * C], wp_d.ap()[kt * P:(kt + 1) * P, :]
            )
        ws["p"] = wp
        ones64 = const.tile([P, HS], dt.bfloat16)
        nc.vector.memset(ones64[:], 1.0)
        rsd_tiles = []
        rsd_engs = [nc.gpsimd, nc.scalar, nc.sync, nc.gpsimd]
        for ct in range(CT):
            rsd = rsdpool.tile([P, NQ], dt.float32, name=f"rsd{ct}", tag=f"rsd{ct}")
            rsd_engs[ct].dma_start(rsd[:], rsdc_d.ap()[ct * P:(ct + 1) * P, :])
            rsd_tiles.append(rsd)

        # ---------------- phase B: QKV GEMMs (fp8 DoubleRow) ----------------
        psctx = ExitStack()
        psB = psctx.enter_context(tc.tile_pool(name="psB", bufs=1, space="PSUM"))

        DR = mybir.MatmulPerfMode.DoubleRow

        # HAM warm-up: the PE idles during the input DMA (~10-20us), so the
        # first QKV matmuls would run at the cold K=4/8 clock for ~3.4us.
        # Keep the PE "busy" with tiny matmuls gated only on the earliest
        # arrivals (ws_q, then xb tile 0) so the clock gate opens before the
        # real GEMMs start.
        warm_ps = psB.tile([P, NQ], dt.float32, tag="gps", name="warm_ps", bufs=4)
        for wi in range(26):
            nc.tensor.matmul(
                warm_ps[:, 0:P], lhsT=ws["q"][:, (wi % 16) * P:(wi % 16 + 1) * P],
                rhs=ws["q"][:, 0:P], start=True, stop=True,
                skip_group_check=True,
            )
        for wi in range(12):
            nc.tensor.matmul(
                warm_ps[:, 0:P], lhsT=xpair[0][:, (wi % 32) * P:(wi % 32 + 1) * P],
                rhs=ws["q"][:, 0:P], start=True, stop=True,
                skip_group_check=True,
            )

        def w_pair(nm, pidx, dtile):
            # [128, 2, 128]: kt in (2*pidx, 2*pidx+1), d-block dtile
            return ws[nm][:].rearrange("p (kt d) -> p kt d", kt=CT)[
                :, 2 * pidx:2 * pidx + 2, dtile * P:(dtile + 1) * P]

        def wfull_pair(nm, pidx):
            return ws[nm][:].rearrange("p (kt d) -> p kt d", kt=CT)[
                :, 2 * pidx:2 * pidx + 2, :]

        def x_pair(pidx, lo, size):
            return xpair[pidx][:].rearrange("p (j n) -> p j n", j=2)[:, :, lo:lo + size]

        cp_flip = 0

        def copy_rr(dst, src):
            nonlocal cp_flip
            cp_flip += 1
            if cp_flip % 2 == 0:
                nc.scalar.copy(dst, src)
            else:
                nc.vector.tensor_copy(dst, src)

        # qT[dtile]: [128, 1024] bf16; ScalarE copy adds the q bias.
        # pidx-outer so each DoubleRow weight load covers both 512-chunks.
        qT = []
        for dtile in range(CT):
            q = qpool.tile([P, NQ], dt.bfloat16, name=f"qT{dtile}")
            ps = psB.tile([P, NQ], dt.float32, name=f"qps{dtile}",
                          tag="gps", bufs=4)
            for pidx in range(2):
                for nch in range(2):
                    nc.tensor.matmul(
                        ps[:, nch * 512:(nch + 1) * 512],
                        lhsT=w_pair("q", pidx, dtile),
                        rhs=x_pair(pidx, nch * 512, 512),
                        start=(pidx == 0), stop=(pidx == 1), perf_mode=DR,
                    )
            if dtile % 2 == 0:
                nc.scalar.activation(
                    q[:], ps[:], ACTF.Identity, bias=qb4[:, dtile:dtile + 1],
                )
            else:
                nc.vector.tensor_scalar(
                    q[:], ps[:], qb4[:, dtile:dtile + 1], None, AOT.add,
                )
            qT.append(q)

        # kT[dtile]: [128, 4096] fp8e4 (stationary operand of QK); pidx-outer
        # in half-dtile groups of 4 chunks so weight loads amortize 4x.
        kT = []
        for dtile in range(CT):
            k = kpool.tile([P, N], dt.float8e4, name=f"kT{dtile}")
            pss = [psB.tile([P, NQ], dt.float32, name=f"kps{dtile}_{i}",
                            tag="gps", bufs=4) for i in range(4)]
            for pidx in range(2):
                for i in range(4):
                    for c in range(2):
                        nch = i * 2 + c
                        nc.tensor.matmul(
                            pss[i][:, c * 512:(c + 1) * 512],
                            lhsT=w_pair("k", pidx, dtile),
                            rhs=x_pair(pidx, nch * 512, 512),
                            start=(pidx == 0), stop=(pidx == 1), perf_mode=DR,
                        )
            for i in range(4):
                copy_rr(k[:, i * NQ:(i + 1) * NQ], pss[i][:])
            kT.append(k)

        # v[mt]: [128 (m), 512 (d over all heads)]
        psctx.close()
        psctx = ExitStack()
        psB = psctx.enter_context(tc.tile_pool(name="psV", bufs=1, space="PSUM"))
        vt2 = []
        for j in range(MT // 2):
            v = vpool.tile([P, 2 * C], dt.bfloat16, name=f"v{j}")
            ps = psB.tile([P, NQ], dt.float32, name=f"vps{j}", tag="vps", bufs=2)
            for sub in range(2):
                mt = 2 * j + sub
                for pidx in range(2):
                    nc.tensor.matmul(
                        ps[:, sub * 512:(sub + 1) * 512],
                        lhsT=x_pair(pidx, mt * P, P),
                        rhs=wfull_pair("v", pidx),
                        start=(pidx == 0), stop=(pidx == 1), perf_mode=DR,
                    )
            copy_rr(v[:], ps[:])
            vt2.append(v)

        def vslice(mt, h):
            return vt2[mt // 2][:, (mt % 2) * C + h * HS:(mt % 2) * C + (h + 1) * HS]

        # ---------------- phase C: attention ----------------
        psctx.close()
        spool = ctx.enter_context(tc.tile_pool(name="spool", bufs=3, space="PSUM"))
        opool = ctx.enter_context(tc.tile_pool(name="opool", bufs=1, space="PSUM"))
        obpool = ctx.enter_context(tc.tile_pool(name="obpool", bufs=1, space="PSUM"))

        # steps: (pair, nch, mt) with mt innermost: oa/ob accumulate one
        # 512-query chunk at a time ([128,512] = 1 PSUM bank each), which
        # frees banks for 3-deep score double-buffering (spool bufs=3) so
        # the QK->exp->QK slot-recycling chain amortizes over 3 steps.
        steps = [
            (hp, mt, nch)
            for hp in range(NH // 2)
            for nch in range(NQ // 512)
            for mt in range(MT)
        ]

        oa_tiles = {}
        ob_tiles = {}
        s_tiles = {}

        def emit_qk(idx):
            hp, mt, nch = steps[idx]
            s = spool.tile([P, NQ], dt.float32, tag="s", name=f"s{idx}")
            kk = kT[hp]
            qq = qT[hp]
            nc.tensor.matmul(
                s[:, 0:512],
                lhsT=kk[0:64, mt * P:(mt + 1) * P],
                rhs=qq[0:64, nch * 512:(nch + 1) * 512],
                start=True, stop=True,
            )
            nc.tensor.matmul(
                s[:, 512:1024],
                lhsT=kk[64:128, mt * P:(mt + 1) * P],
                rhs=qq[64:128, nch * 512:(nch + 1) * 512],
                start=True, stop=True,
            )
            s_tiles[idx] = s

        e_tiles = {}
        on_all = onpool.tile([P, CT * NQ], dt.float8e4, name="on_all")
        exp_ctr = 0

        def emit_exp(idx):
            nonlocal exp_ctr
            s = s_tiles.pop(idx)
            kind = EXPSCHED[exp_ctr % len(EXPSCHED)]
            exp_ctr += 1
            if kind == "A":
                e = epool.tile([P, NQ], dt.bfloat16, name=f"e{idx}",
                               tag="e", bufs=4)
                nc.scalar.activation(e[:], s[:], ACTF.Exp, scale=0.125)
            else:
                e = epool.tile([P, NQ], dt.int16, name=f"e{idx}",
                               tag="e", bufs=4)
                nc.vector.tensor_scalar(
                    e[:], s[:], SCH_A16 * 0.125, SCH_B16, AOT.mult, AOT.add
                )
            e_tiles[idx] = e

        def as_bf16(e, half):
            sl = e[:, half * 512:(half + 1) * 512]
            return sl if e.tensor.dtype == dt.bfloat16 else sl.bitcast(dt.bfloat16)

        def emit_pv(idx):
            hp, mt, nch = steps[idx]
            e = e_tiles[idx]
            oa = oa_tiles[(hp, nch)]
            h0, h1 = 2 * hp, 2 * hp + 1
            first = mt == 0
            last = mt == MT - 1
            nc.tensor.matmul(
                oa[0:64, :], lhsT=vslice(mt, h0),
                rhs=as_bf16(e, 0), start=first, stop=last, skip_group_check=True,
            )
            nc.tensor.matmul(
                oa[64:128, :], lhsT=vslice(mt, h1),
                rhs=as_bf16(e, 1), start=first, stop=last,
                skip_group_check=True,
            )

        def emit_pv_ones(idx):
            hp, mt, nch = steps[idx]
            e = e_tiles.pop(idx)
            ob = ob_tiles[(hp, nch)]
            first = mt == 0
            last = mt == MT - 1
            nc.tensor.matmul(
                ob[0:64, :], lhsT=ones64[:], rhs=as_bf16(e, 0),
                start=first, stop=last, skip_group_check=True,
            )
            nc.tensor.matmul(
                ob[64:128, :], lhsT=ones64[:], rhs=as_bf16(e, 1),
                start=first, stop=last, skip_group_check=True,
            )
            if last:
                # normalize this 512-query chunk: o_norm = oA * recip_fast(oB)
                oa = oa_tiles[(hp, nch)]
                r = rpool.tile([P, 512], dt.float32, name=f"r{hp}_{nch}", tag="r")
                nc.vector.reciprocal_approx_fast(r[:], ob[:])
                nc.vector.tensor_mul(
                    on_all[:, hp * NQ + nch * 512:hp * NQ + (nch + 1) * 512],
                    oa[:], r[:])
                del oa_tiles[(hp, nch)], ob_tiles[(hp, nch)]

        NSS = len(steps) // 2
        emit_qk(0)
        emit_qk(1)
        for ss in range(NSS + 1):
            if ss < NSS:
                for st in (2 * ss, 2 * ss + 1):
                    hp, mt, nch = steps[st]
                    if mt == 0 and (hp, nch) not in oa_tiles:
                        oa_tiles[(hp, nch)] = opool.tile(
                            [P, 512], dt.float32, tag="oa", name=f"oa{hp}_{nch}")
                        ob_tiles[(hp, nch)] = obpool.tile(
                            [P, 512], dt.float32, tag="ob", name=f"ob{hp}_{nch}")
                emit_exp(2 * ss)
                emit_exp(2 * ss + 1)
            if ss > 0:
                emit_pv(2 * (ss - 1))
                emit_pv(2 * (ss - 1) + 1)
                emit_pv_ones(2 * (ss - 1))
                emit_pv_ones(2 * (ss - 1) + 1)
            if 2 * (ss + 1) < len(steps):
                emit_qk(2 * (ss + 1))
            if 2 * (ss + 1) + 1 < len(steps):
                emit_qk(2 * (ss + 1) + 1)

        # ---------------- phase D: proj + residual + out ----------------
        on_r = on_all[:].rearrange("p (kt q) -> p kt q", kt=CT)
        for ct in range(CT):
            # reuse the s-slots (freed as the tail pairs finish) so proj
            # overlaps the last accumulation group
            ys = spool.tile([P, NQ], dt.float32, tag="s", name=f"yps{ct}")
            for nch in range(NQ // 512):
                for pidx in range(2):
                    nc.tensor.matmul(
                        ys[:, nch * 512:(nch + 1) * 512],
                        lhsT=w_pair("p", pidx, ct),
                        rhs=on_r[:, 2 * pidx:2 * pidx + 2,
                                 nch * 512:(nch + 1) * 512],
                        start=(pidx == 0), stop=(pidx == 1), perf_mode=DR,
                    )
            ot = outpool.tile([P, NQ], dt.float32, name=f"ot{ct}", tag="ot")
            # y + (cout + resid), DMA each half as soon as it's done
            nc.vector.tensor_add(ot[:], ys[:], rsd_tiles[ct][:])
            for nch in range(NQ // 512):
                half = slice(nch * 512, (nch + 1) * 512)
                deng = nc.sync if (2 * ct + nch) % 2 == 0 else nc.scalar
                deng.dma_start(y_d.ap()[ct * P:(ct + 1) * P, half], ot[:, half])

    nc.compile()
    return nc


def make_in_maps(inputs):
    x = np.asarray(inputs["x"], dtype=np.float32).reshape(2, C, N)
    gn_w = np.asarray(inputs["gn_w"], np.float32)
    gn_b = np.asarray(inputs["gn_b"], np.float32)
    wq_w = np.asarray(inputs["wq_w"], np.float32)
    wk_w = np.asarray(inputs["wk_w"], np.float32)
    wv_w = np.asarray(inputs["wv_w"], np.float32)
    wp_w = np.asarray(inputs["proj_w"], np.float32)
    wq_b = np.asarray(inputs["wq_b"], np.float32)
    wv_b = np.asarray(inputs["wv_b"], np.float32)
    pj_b = np.asarray(inputs["proj_b"], np.float32)

    def t4(v):
        return np.ascontiguousarray(np.asarray(v, np.float32).reshape(CT, P).T)

    G = 32
    wpT = np.ascontiguousarray(wp_w.T).astype(FP8)
    per_batch = []
    for b in range(2):
        xg = x[b].reshape(G, C // G * N)
        mu = xg.mean(axis=1)
        var = xg.var(axis=1)
        a = gn_w * np.repeat(1.0 / np.sqrt(var + EPS), C // G)
        bb = gn_b - np.repeat(mu, C // G) * a
        wsq = np.ascontiguousarray(a[:, None] * wq_w.T).astype(FP8)
        wsk = np.ascontiguousarray(a[:, None] * wk_w.T).astype(FP8)
        wsv = np.ascontiguousarray(a[:, None] * wv_w.T).astype(FP8)
        qbias = wq_w @ bb + wq_b
        vb = wv_w @ bb + wv_b
        co = wp_w @ vb + pj_b
        per_batch.append(dict(
            wsq=wsq, wsk=wsk, wsv=wsv, wpT=wpT,
            qb4=t4(qbias), cout=co,
        ))

    in_maps = []
    for core in range(8):
        b, r = core // 4, core % 4
        nq0 = r * NQ
        rolled = np.roll(x[b], -nq0, axis=1)
        m = dict(per_batch[b])
        co = m.pop("cout")
        m["xb"] = rolled.astype(FP8)
        # residual + cout pre-added on the host
        m["rsdc"] = np.ascontiguousarray(x[b][:, nq0:nq0 + NQ] + co[:, None])
        in_maps.append(m)
    return in_maps


def assemble(results):
    out = np.empty((2, C, N), np.float32)
    for core in range(8):
        b, r = core // 4, core % 4
        out[b][:, r * NQ:(r + 1) * NQ] = results[core]["y"]
    return out.reshape(2, C, 64, 64)


def get_program():
    if "nc" not in _CACHE:
        _CACHE["nc"] = build_program()
    return _CACHE["nc"]


def kernel(**inputs):
    nc = get_program()
    in_maps = make_in_maps(inputs)
    res = run_bass_kernel_spmd(nc, in_maps, core_ids=list(range(8)))
    return assemble(res.results)


# revision 16
# speedup vs baseline: 1.0257x; 1.0257x over previous
"""Trainium2 Bass kernel for nn_AttentionBlock (GroupNorm + 8-head self-attention + proj + residual).

Full inputs in, full output out. Sharding: 8 cores = 2 batches x 4-way split of
the 4096 query pixels. Each core runs an identical SPMD program on per-core
input data (x rolled so its 1024 query pixels sit first; attention and
groupnorm are permutation-invariant over keys/pixels, so rotation is exact).

Host-side folding (exact, fp32): groupnorm h = a(.)x + b folds into the QKV
weights: ws_q = diag(a) Wq^T (fp8) with qbias = Wq b + wq_b; k's constant
cancels in softmax; v's constant rides through (cout = proj_w (Wv b + wv_b) +
proj_b added at the end). The residual+cout constant is pre-added on the host
into the rsdc tensor.

Device-side:
  QKV GEMMs in fp8 DoubleRow, pidx-outer so each stationary weight load
  covers several matmuls.  Attention: S^T tiles (keys on partitions); QK is
  emitted as four 64x64-quadrant matmuls (explicit tile_position) so all four
  run concurrently in the PE array; PV and the ones (denominator) matmuls are
  column-tiled pairs.  exp is round-robined over ScalarE (exact, 1/8 scale
  fused), Pool/GpSimd and DVE (int16-Schraudolph); o_norm = oA * recip(oB);
  y^T = proj_w o_norm^T + rsdc, DMA'd out per 512-chunk.
"""

import numpy as np
import ml_dtypes
from contextlib import ExitStack

import concourse.bacc as bacc
import concourse.tile as tile
import concourse.mybir as mybir
from concourse.bass_utils import run_bass_kernel_spmd

BF16 = ml_dtypes.bfloat16
FP8 = ml_dtypes.float8_e4m3
F32 = np.float32

P = 128          # partitions
C = 512          # channels
NH = 8
HS = 64
N = 4096         # pixels (keys)
NQ = 1024        # queries per core
CT = 4           # channel tiles of 128
MT = 32          # m (key) tiles of 128
EPS = 1e-5
SCH_A16 = float(2.0 ** 7 / np.log(2.0))   # int16 Schraudolph exp
SCH_B16 = 16250.4062

dt = mybir.dt
AOT = mybir.AluOpType
ACTF = mybir.ActivationFunctionType
AXT = mybir.AxisListType

_CACHE = {}

# exp engine schedule per score tile: P=pool/gpsimd, A=scalar-exact, D=dve
EXPSCHED = "AD"


def build_program():
    nc = bacc.Bacc("TRN2", target_bir_lowering=False, debug=False, num_devices=8)

    xb_d = nc.dram_tensor("xb", [C, N], dt.float8e4, kind="ExternalInput")
    rsdc_d = nc.dram_tensor("rsdc", [C, NQ], dt.float32, kind="ExternalInput")
    wsq_d = nc.dram_tensor("wsq", [C, C], dt.float8e4, kind="ExternalInput")
    wsk_d = nc.dram_tensor("wsk", [C, C], dt.float8e4, kind="ExternalInput")
    wsv_d = nc.dram_tensor("wsv", [C, C], dt.float8e4, kind="ExternalInput")
    wp_d = nc.dram_tensor("wpT", [C, C], dt.float8e4, kind="ExternalInput")
    qb_d = nc.dram_tensor("qb4", [P, CT], dt.float32, kind="ExternalInput")
    y_d = nc.dram_tensor("y", [C, NQ], dt.float32, kind="ExternalOutput")

    with tile.TileContext(nc) as tc, ExitStack() as ctx:
        const = ctx.enter_context(tc.tile_pool(name="const", bufs=1))
        wpool = ctx.enter_context(tc.tile_pool(name="wpool", bufs=1))
        xpool = ctx.enter_context(tc.tile_pool(name="xpool", bufs=1))
        kpool = ctx.enter_context(tc.tile_pool(name="kpool", bufs=1))
        qpool = ctx.enter_context(tc.tile_pool(name="qpool", bufs=1))
        vpool = ctx.enter_context(tc.tile_pool(name="vpool", bufs=1))
        epool = ctx.enter_context(tc.tile_pool(name="epool", bufs=3))
        onpool = ctx.enter_context(tc.tile_pool(name="onpool", bufs=1))
        rpool = ctx.enter_context(tc.tile_pool(name="rpool", bufs=2))
        rsdpool = ctx.enter_context(tc.tile_pool(name="rsdpool", bufs=2))
        outpool = ctx.enter_context(tc.tile_pool(name="outpool", bufs=2))

        # ---------------- input DMA ----------------
        # tiny fp8 q/k weights first (they gate the first GEMMs), then the
        # 2MB xb split over four queues; wsv/qb next (gate V/q-bias); wp and
        # the residual go last (needed only at the tail).
        ws = {}
        w = wpool.tile([P, CT * C], dt.float8e4, tag="ws_q", name="ws_q")
        for kt in range(CT):
            nc.sync.dma_start(
                w[:, kt * C:(kt + 1) * C], wsq_d.ap()[kt * P:(kt + 1) * P, :]
            )
        ws["q"] = w

        xpair = [xpool.tile([P, 2 * N], dt.float8e4, name=f"xp{pidx}")
                 for pidx in range(2)]
        for t, eng in ((0, nc.sync), (1, nc.scalar), (2, nc.gpsimd), (3, nc.gpsimd)):
            eng.dma_start(
                xpair[t // 2][:, (t % 2) * N:(t % 2 + 1) * N],
                xb_d.ap()[t * P:(t + 1) * P, :],
            )

        w = wpool.tile([P, CT * C], dt.float8e4, tag="ws_k", name="ws_k")
        for kt in range(CT):
            nc.scalar.dma_start(
                w[:, kt * C:(kt + 1) * C], wsk_d.ap()[kt * P:(kt + 1) * P, :]
            )
        ws["k"] = w
        qb4 = const.tile([P, CT], dt.float32)
        nc.sync.dma_start(qb4[:], qb_d.ap())
        wsv = wpool.tile([P, CT * C], dt.float8e4, tag="ws_v", name="ws_v")
        for kt in range(CT):
            eng = nc.scalar if kt % 2 == 0 else nc.sync
            eng.dma_start(
                wsv[:, kt * C:(kt + 1) * C], wsv_d.ap()[kt * P:(kt + 1) * P, :]
            )
        ws["v"] = wsv
        wp = wpool.tile([P, CT * C], dt.float8e4, tag="w_p", name="w_p")
        for kt in range(CT):
            eng = nc.gpsimd if kt % 2 == 0 else nc.sync
            eng.dma_start(
                wp[:, kt * C:(kt + 1) * C], wp_d.ap()[kt * P:(kt + 1) * P, :]
            )
        ws["p"] = wp
        ones64 = const.tile([P, HS], dt.bfloat16)
        nc.vector.memset(ones64[:], 1.0)
        rsd_tiles = []
        rsd_engs = [nc.gpsimd, nc.scalar, nc.sync, nc.gpsimd]
        for ct in range(CT):
            rsd = rsdpool.tile([P, NQ], dt.float32, name=f"rsd{ct}", tag=f"rsd{ct}")
            rsd_engs[ct].dma_start(rsd[:], rsdc_d.ap()[ct * P:(ct + 1) * P, :])
            rsd_tiles.append(rsd)

        # ---------------- phase B: QKV GEMMs (fp8 DoubleRow) ----------------
        psctx = ExitStack()
        psB = psctx.enter_context(tc.tile_pool(name="psB", bufs=1, space="PSUM"))

        DR = mybir.MatmulPerfMode.DoubleRow

        # HAM warm-up: the PE idles during the input DMA (~10-20us), so the
        # first QKV matmuls would run at the cold K=4/8 clock for ~3.4us.
        # Keep the PE "busy" with tiny matmuls gated only on the earliest
        # arrivals (ws_q, then xb tile 0) so the clock gate opens before the
        # real GEMMs start.
        warm_ps = psB.tile([P, NQ], dt.float32, tag="gps", name="warm_ps", bufs=4)
        for wi in range(14):
            nc.tensor.matmul(
                warm_ps[:, 0:P], lhsT=ws["q"][:, (wi % 16) * P:(wi % 16 + 1) * P],
                rhs=ws["q"][:, 0:P], start=True, stop=True,
                skip_group_check=True,
            )
        for wi in range(10):
            nc.tensor.matmul(
                warm_ps[:, 0:P], lhsT=xpair[0][:, (wi % 32) * P:(wi % 32 + 1) * P],
                rhs=ws["q"][:, 0:P], start=True, stop=True,
                skip_group_check=True,
            )

        def w_pair(nm, pidx, dtile):
            # [128, 2, 128]: kt in (2*pidx, 2*pidx+1), d-block dtile
            return ws[nm][:].rearrange("p (kt d) -> p kt d", kt=CT)[
                :, 2 * pidx:2 * pidx + 2, dtile * P:(dtile + 1) * P]

        def wfull_pair(nm, pidx):
            return ws[nm][:].rearrange("p (kt d) -> p kt d", kt=CT)[
                :, 2 * pidx:2 * pidx + 2, :]

        def x_pair(pidx, lo, size):
            return xpair[pidx][:].rearrange("p (j n) -> p j n", j=2)[:, :, lo:lo + size]

        cp_flip = 0

        def copy_rr(dst, src):
            nonlocal cp_flip
            cp_flip += 1
            if cp_flip % 2 == 0:
                nc.scalar.copy(dst, src)
            else:
                nc.vector.tensor_copy(dst, src)

        # qT[dtile]: [128, 1024] bf16; ScalarE copy adds the q bias.
        # pidx-outer so each DoubleRow weight load covers both 512-chunks.
        qT = []
        for dtile in range(CT):
            q = qpool.tile([P, NQ], dt.bfloat16, name=f"qT{dtile}")
            ps = psB.tile([P, NQ], dt.float32, name=f"qps{dtile}",
                          tag="gps", bufs=4)
            for pidx in range(2):
                for nch in range(2):
                    nc.tensor.matmul(
                        ps[:, nch * 512:(nch + 1) * 512],
                        lhsT=w_pair("q", pidx, dtile),
                        rhs=x_pair(pidx, nch * 512, 512),
                        start=(pidx == 0), stop=(pidx == 1), perf_mode=DR,
                    )
            if dtile % 2 == 0:
                nc.scalar.activation(
                    q[:], ps[:], ACTF.Identity, bias=qb4[:, dtile:dtile + 1],
                )
            else:
                nc.vector.tensor_scalar(
                    q[:], ps[:], qb4[:, dtile:dtile + 1], None, AOT.add,
                )
            qT.append(q)

        # kT[dtile]: [128, 4096] fp8e4 (stationary operand of QK); pidx-outer
        # in half-dtile groups of 4 chunks so weight loads amortize 4x.
        kT = []
        for dtile in range(CT):
            k = kpool.tile([P, N], dt.float8e4, name=f"kT{dtile}")
            pss = [psB.tile([P, NQ], dt.float32, name=f"kps{dtile}_{i}",
                            tag="gps", bufs=4) for i in range(4)]
            for pidx in range(2):
                for i in range(4):
                    for c in range(2):
                        nch = i * 2 + c
                        nc.tensor.matmul(
                            pss[i][:, c * 512:(c + 1) * 512],
                            lhsT=w_pair("k", pidx, dtile),
                            rhs=x_pair(pidx, nch * 512, 512),
                            start=(pidx == 0), stop=(pidx == 1), perf_mode=DR,
                        )
            for i in range(4):
                copy_rr(k[:, i * NQ:(i + 1) * NQ], pss[i][:])
            kT.append(k)

        # v[mt]: [128 (m), 512 (d over all heads)]
        vt2 = []
        for j in range(MT // 2):
            v = vpool.tile([P, 2 * C], dt.bfloat16, name=f"v{j}")
            ps = psB.tile([P, NQ], dt.float32, name=f"vps{j}", tag="gps", bufs=4)
            for sub in range(2):
                mt = 2 * j + sub
                for pidx in range(2):
                    nc.tensor.matmul(
                        ps[:, sub * 512:(sub + 1) * 512],
                        lhsT=x_pair(pidx, mt * P, P),
                        rhs=wfull_pair("v", pidx),
                        start=(pidx == 0), stop=(pidx == 1), perf_mode=DR,
                    )
            copy_rr(v[:], ps[:])
            vt2.append(v)

        def vslice(mt, h):
            return vt2[mt // 2][:, (mt % 2) * C + h * HS:(mt % 2) * C + (h + 1) * HS]

        # ---------------- phase C: attention ----------------
        psctx.close()
        spool = ctx.enter_context(tc.tile_pool(name="spool", bufs=3, space="PSUM"))
        opool = ctx.enter_context(tc.tile_pool(name="opool", bufs=1, space="PSUM"))
        obpool = ctx.enter_context(tc.tile_pool(name="obpool", bufs=1, space="PSUM"))

        # steps: (pair, nch, mt) with mt innermost: oa/ob accumulate one
        # 512-query chunk at a time ([128,512] = 1 PSUM bank each), which
        # frees banks for 3-deep score double-buffering (spool bufs=3) so
        # the QK->exp->QK slot-recycling chain amortizes over 3 steps.
        steps = [
            (hp, mt, nch)
            for hp in range(NH // 2)
            for nch in range(NQ // 512)
            for mt in range(MT)
        ]

        oa_tiles = {}
        ob_tiles = {}
        s_tiles = {}

        def emit_qk(idx):
            hp, mt, nch = steps[idx]
            s = spool.tile([P, NQ], dt.float32, tag="s", name=f"s{idx}")
            kk = kT[hp]
            qq = qT[hp]
            nc.tensor.matmul(
                s[:, 0:512],
                lhsT=kk[0:64, mt * P:(mt + 1) * P],
                rhs=qq[0:64, nch * 512:(nch + 1) * 512],
                start=True, stop=True,
            )
            nc.tensor.matmul(
                s[:, 512:1024],
                lhsT=kk[64:128, mt * P:(mt + 1) * P],
                rhs=qq[64:128, nch * 512:(nch + 1) * 512],
                start=True, stop=True,
            )
            s_tiles[idx] = s

        e_tiles = {}
        on_all = onpool.tile([P, CT * NQ], dt.float8e4, name="on_all")
        exp_ctr = 0

        def emit_exp(idx):
            nonlocal exp_ctr
            s = s_tiles.pop(idx)
            kind = EXPSCHED[exp_ctr % len(EXPSCHED)]
            exp_ctr += 1
            if kind == "A":
                e = epool.tile([P, NQ], dt.bfloat16, name=f"e{idx}",
                               tag="e", bufs=4)
                nc.scalar.activation(e[:], s[:], ACTF.Exp, scale=0.125)
            else:
                e = epool.tile([P, NQ], dt.int16, name=f"e{idx}",
                               tag="e", bufs=4)
                nc.vector.tensor_scalar(
                    e[:], s[:], SCH_A16 * 0.125, SCH_B16, AOT.mult, AOT.add
                )
            e_tiles[idx] = e

        def as_bf16(e, half):
            sl = e[:, half * 512:(half + 1) * 512]
            return sl if e.tensor.dtype == dt.bfloat16 else sl.bitcast(dt.bfloat16)

        def emit_pv(idx):
            hp, mt, nch = steps[idx]
            e = e_tiles[idx]
            oa = oa_tiles[(hp, nch)]
            h0, h1 = 2 * hp, 2 * hp + 1
            first = mt == 0
            last = mt == MT - 1
            nc.tensor.matmul(
                oa[0:64, :], lhsT=vslice(mt, h0),
                rhs=as_bf16(e, 0), start=first, stop=last, skip_group_check=True,
            )
            nc.tensor.matmul(
                oa[64:128, :], lhsT=vslice(mt, h1),
                rhs=as_bf16(e, 1), start=first, stop=last,
                skip_group_check=True,
            )

        def emit_pv_ones(idx):
            hp, mt, nch = steps[idx]
            e = e_tiles.pop(idx)
            ob = ob_tiles[(hp, nch)]
            first = mt == 0
            last = mt == MT - 1
            nc.tensor.matmul(
                ob[0:64, :], lhsT=ones64[:], rhs=as_bf16(e, 0),
                start=first, stop=last, skip_group_check=True,
            )
            nc.tensor.matmul(
                ob[64:128, :], lhsT=ones64[:], rhs=as_bf16(e, 1),
                start=first, stop=last, skip_group_check=True,
            )
            if last:
                # normalize this 512-query chunk: o_norm = oA * recip_fast(oB)
                oa = oa_tiles[(hp, nch)]
                r = rpool.tile([P, 512], dt.float32, name=f"r{hp}_{nch}", tag="r")
                nc.vector.reciprocal_approx_fast(r[:], ob[:])
                nc.vector.tensor_mul(
                    on_all[:, hp * NQ + nch * 512:hp * NQ + (nch + 1) * 512],
                    oa[:], r[:])
                del oa_tiles[(hp, nch)], ob_tiles[(hp, nch)]

        NSS = len(steps) // 2
        emit_qk(0)
        emit_qk(1)
        for ss in range(NSS + 1):
            if ss < NSS:
                for st in (2 * ss, 2 * ss + 1):
                    hp, mt, nch = steps[st]
                    if mt == 0 and (hp, nch) not in oa_tiles:
                        oa_tiles[(hp, nch)] = opool.tile(
                            [P, 512], dt.float32, tag="oa", name=f"oa{hp}_{nch}")
                        ob_tiles[(hp, nch)] = obpool.tile(
                            [P, 512], dt.float32, tag="ob", name=f"ob{hp}_{nch}")
                emit_exp(2 * ss)
                emit_exp(2 * ss + 1)
            if ss > 0:
                emit_pv(2 * (ss - 1))
                emit_pv(2 * (ss - 1) + 1)
                emit_pv_ones(2 * (ss - 1))
                emit_pv_ones(2 * (ss - 1) + 1)
            if 2 * (ss + 1) < len(steps):
                emit_qk(2 * (ss + 1))
            if 2 * (ss + 1) + 1 < len(steps):
                emit_qk(2 * (ss + 1) + 1)

        # ---------------- phase D: proj + residual + out ----------------
        on_r = on_all[:].rearrange("p (kt q) -> p kt q", kt=CT)
        for ct in range(CT):
            # reuse the s-slots (freed as the tail pairs finish) so proj
            # overlaps the last accumulation group
            ys = spool.tile([P, NQ], dt.float32, tag="s", name=f"yps{ct}")
            for nch in range(NQ // 512):
                for pidx in range(2):
                    nc.tensor.matmul(
                        ys[:, nch * 512:(nch + 1) * 512],
                        lhsT=w_pair("p", pidx, ct),
                        rhs=on_r[:, 2 * pidx:2 * pidx + 2,
                                 nch * 512:(nch + 1) * 512],
                        start=(pidx == 0), stop=(pidx == 1), perf_mode=DR,
                    )
            ot = outpool.tile([P, NQ], dt.float32, name=f"ot{ct}", tag="ot")
            # y + (cout + resid), DMA each half as soon as it's done
            nc.vector.tensor_add(ot[:], ys[:], rsd_tiles[ct][:])
            for nch in range(NQ // 512):
                half = slice(nch * 512, (nch + 1) * 512)
                deng = nc.sync if (2 * ct + nch) % 2 == 0 else nc.scalar
                deng.dma_start(y_d.ap()[ct * P:(ct + 1) * P, half], ot[:, half])

    nc.compile()
    return nc


def make_in_maps(inputs):
    x = np.asarray(inputs["x"], dtype=np.float32).reshape(2, C, N)
    gn_w = np.asarray(inputs["gn_w"], np.float32)
    gn_b = np.asarray(inputs["gn_b"], np.float32)
    wq_w = np.asarray(inputs["wq_w"], np.float32)
    wk_w = np.asarray(inputs["wk_w"], np.float32)
    wv_w = np.asarray(inputs["wv_w"], np.float32)
    wp_w = np.asarray(inputs["proj_w"], np.float32)
    wq_b = np.asarray(inputs["wq_b"], np.float32)
    wv_b = np.asarray(inputs["wv_b"], np.float32)
    pj_b = np.asarray(inputs["proj_b"], np.float32)

    def t4(v):
        return np.ascontiguousarray(np.asarray(v, np.float32).reshape(CT, P).T)

    G = 32
    wpT = np.ascontiguousarray(wp_w.T).astype(FP8)
    per_batch = []
    for b in range(2):
        xg = x[b].reshape(G, C // G * N)
        mu = xg.mean(axis=1)
        var = xg.var(axis=1)
        a = gn_w * np.repeat(1.0 / np.sqrt(var + EPS), C // G)
        bb = gn_b - np.repeat(mu, C // G) * a
        wsq = np.ascontiguousarray(a[:, None] * wq_w.T).astype(FP8)
        wsk = np.ascontiguousarray(a[:, None] * wk_w.T).astype(FP8)
        wsv = np.ascontiguousarray(a[:, None] * wv_w.T).astype(FP8)
        qbias = wq_w @ bb + wq_b
        vb = wv_w @ bb + wv_b
        co = wp_w @ vb + pj_b
        per_batch.append(dict(
            wsq=wsq, wsk=wsk, wsv=wsv, wpT=wpT,
            qb4=t4(qbias), cout=co,
        ))

    in_maps = []
    for core in range(8):
        b, r = core // 4, core % 4
        nq0 = r * NQ
        rolled = np.roll(x[b], -nq0, axis=1)
        m = dict(per_batch[b])
        co = m.pop("cout")
        m["xb"] = rolled.astype(FP8)
        # residual + cout pre-added on the host
        m["rsdc"] = np.ascontiguousarray(x[b][:, nq0:nq0 + NQ] + co[:, None])
        in_maps.append(m)
    return in_maps


def assemble(results):
    out = np.empty((2, C, N), np.float32)
    for core in range(8):
        b, r = core // 4, core % 4
        out[b][:, r * NQ:(r + 1) * NQ] = results[core]["y"]
    return out.reshape(2, C, 64, 64)


def get_program():
    if "nc" not in _CACHE:
        _CACHE["nc"] = build_program()
    return _CACHE["nc"]


def kernel(**inputs):
    nc = get_program()
    in_maps = make_in_maps(inputs)
    res = run_bass_kernel_spmd(nc, in_maps, core_ids=list(range(8)))
    return assemble(res.results)


# revision 17
# speedup vs baseline: 1.0318x; 1.0060x over previous
"""Trainium2 Bass kernel for nn_AttentionBlock (GroupNorm + 8-head self-attention + proj + residual).

Full inputs in, full output out. Sharding: 8 cores = 2 batches x 4-way split of
the 4096 query pixels. Each core runs an identical SPMD program on per-core
input data (x rolled so its 1024 query pixels sit first; attention and
groupnorm are permutation-invariant over keys/pixels, so rotation is exact).

Host-side folding (exact, fp32): groupnorm h = a(.)x + b folds into the QKV
weights: ws_q = diag(a) Wq^T (fp8) with qbias = Wq b + wq_b; k's constant
cancels in softmax; v's constant rides through (cout = proj_w (Wv b + wv_b) +
proj_b added at the end). The residual+cout constant is pre-added on the host
into the rsdc tensor.

Device-side:
  QKV GEMMs in fp8 DoubleRow, pidx-outer so each stationary weight load
  covers several matmuls.  Attention: S^T tiles (keys on partitions); QK is
  emitted as four 64x64-quadrant matmuls (explicit tile_position) so all four
  run concurrently in the PE array; PV and the ones (denominator) matmuls are
  column-tiled pairs.  exp is round-robined over ScalarE (exact, 1/8 scale
  fused), Pool/GpSimd and DVE (int16-Schraudolph); o_norm = oA * recip(oB);
  y^T = proj_w o_norm^T + rsdc, DMA'd out per 512-chunk.
"""

import numpy as np
import ml_dtypes
from contextlib import ExitStack

import concourse.bacc as bacc
import concourse.tile as tile
import concourse.mybir as mybir
from concourse.bass_utils import run_bass_kernel_spmd

BF16 = ml_dtypes.bfloat16
FP8 = ml_dtypes.float8_e4m3
F32 = np.float32

P = 128          # partitions
C = 512          # channels
NH = 8
HS = 64
N = 4096         # pixels (keys)
NQ = 1024        # queries per core
CT = 4           # channel tiles of 128
MT = 32          # m (key) tiles of 128
EPS = 1e-5
SCH_A16 = float(2.0 ** 7 / np.log(2.0))   # int16 Schraudolph exp
SCH_B16 = 16250.4062

dt = mybir.dt
AOT = mybir.AluOpType
ACTF = mybir.ActivationFunctionType
AXT = mybir.AxisListType

_CACHE = {}

# exp engine schedule per score tile: P=pool/gpsimd, A=scalar-exact, D=dve
EXPSCHED = "AD"


def build_program():
    nc = bacc.Bacc("TRN2", target_bir_lowering=False, debug=False, num_devices=8)

    xb_d = nc.dram_tensor("xb", [C, N], dt.float8e4, kind="ExternalInput")
    rsdc_d = nc.dram_tensor("rsdc", [C, NQ], dt.float32, kind="ExternalInput")
    wsq_d = nc.dram_tensor("wsq", [C, C], dt.float8e4, kind="ExternalInput")
    wsk_d = nc.dram_tensor("wsk", [C, C], dt.float8e4, kind="ExternalInput")
    wsv_d = nc.dram_tensor("wsv", [C, C], dt.float8e4, kind="ExternalInput")
    wp_d = nc.dram_tensor("wpT", [C, C], dt.float8e4, kind="ExternalInput")
    qb_d = nc.dram_tensor("qb4", [P, CT], dt.float32, kind="ExternalInput")
    y_d = nc.dram_tensor("y", [C, NQ], dt.float32, kind="ExternalOutput")

    with tile.TileContext(nc) as tc, ExitStack() as ctx:
        const = ctx.enter_context(tc.tile_pool(name="const", bufs=1))
        wpool = ctx.enter_context(tc.tile_pool(name="wpool", bufs=1))
        xpool = ctx.enter_context(tc.tile_pool(name="xpool", bufs=1))
        kpool = ctx.enter_context(tc.tile_pool(name="kpool", bufs=1))
        qpool = ctx.enter_context(tc.tile_pool(name="qpool", bufs=1))
        vpool = ctx.enter_context(tc.tile_pool(name="vpool", bufs=1))
        epool = ctx.enter_context(tc.tile_pool(name="epool", bufs=3))
        onpool = ctx.enter_context(tc.tile_pool(name="onpool", bufs=1))
        rpool = ctx.enter_context(tc.tile_pool(name="rpool", bufs=2))
        rsdpool = ctx.enter_context(tc.tile_pool(name="rsdpool", bufs=2))
        outpool = ctx.enter_context(tc.tile_pool(name="outpool", bufs=4))

        # ---------------- input DMA ----------------
        # tiny fp8 q/k weights first (they gate the first GEMMs), then the
        # 2MB xb split over four queues; wsv/qb next (gate V/q-bias); wp and
        # the residual go last (needed only at the tail).
        ws = {}
        w = wpool.tile([P, CT * C], dt.float8e4, tag="ws_q", name="ws_q")
        for kt in range(CT):
            nc.sync.dma_start(
                w[:, kt * C:(kt + 1) * C], wsq_d.ap()[kt * P:(kt + 1) * P, :]
            )
        ws["q"] = w

        xpair = [xpool.tile([P, 2 * N], dt.float8e4, name=f"xp{pidx}")
                 for pidx in range(2)]
        for t, eng in ((0, nc.sync), (1, nc.scalar), (2, nc.gpsimd)):
            eng.dma_start(
                xpair[t // 2][:, (t % 2) * N:(t % 2 + 1) * N],
                xb_d.ap()[t * P:(t + 1) * P, :],
            )
        nc.sync.dma_start(xpair[1][:, N:N + 2048], xb_d.ap()[3 * P:4 * P, 0:2048])
        nc.scalar.dma_start(xpair[1][:, N + 2048:2 * N],
                            xb_d.ap()[3 * P:4 * P, 2048:N])

        w = wpool.tile([P, CT * C], dt.float8e4, tag="ws_k", name="ws_k")
        for kt in range(CT):
            nc.scalar.dma_start(
                w[:, kt * C:(kt + 1) * C], wsk_d.ap()[kt * P:(kt + 1) * P, :]
            )
        ws["k"] = w
        qb4 = const.tile([P, CT], dt.float32)
        nc.sync.dma_start(qb4[:], qb_d.ap())
        wsv = wpool.tile([P, CT * C], dt.float8e4, tag="ws_v", name="ws_v")
        for kt in range(CT):
            eng = nc.scalar if kt % 2 == 0 else nc.sync
            eng.dma_start(
                wsv[:, kt * C:(kt + 1) * C], wsv_d.ap()[kt * P:(kt + 1) * P, :]
            )
        ws["v"] = wsv
        wp = wpool.tile([P, CT * C], dt.float8e4, tag="w_p", name="w_p")
        for kt in range(CT):
            eng = nc.gpsimd if kt % 2 == 0 else nc.sync
            eng.dma_start(
                wp[:, kt * C:(kt + 1) * C], wp_d.ap()[kt * P:(kt + 1) * P, :]
            )
        ws["p"] = wp
        ones64 = const.tile([P, HS], dt.bfloat16)
        nc.vector.memset(ones64[:], 1.0)

        # ---------------- phase B: QKV GEMMs (fp8 DoubleRow) ----------------
        psctx = ExitStack()
        psB = psctx.enter_context(tc.tile_pool(name="psB", bufs=1, space="PSUM"))

        DR = mybir.MatmulPerfMode.DoubleRow

        # HAM warm-up: the PE idles during the input DMA (~10-20us), so the
        # first QKV matmuls would run at the cold K=4/8 clock for ~3.4us.
        # Keep the PE "busy" with tiny matmuls gated only on the earliest
        # arrivals (ws_q, then xb tile 0) so the clock gate opens before the
        # real GEMMs start.
        warm_ps = psB.tile([P, NQ], dt.float32, tag="gps", name="warm_ps", bufs=4)
        for wi in range(8):
            nc.tensor.matmul(
                warm_ps[:, 0:P], lhsT=ws["q"][:, (wi % 4) * P:(wi % 4 + 1) * P],
                rhs=ws["q"][:, 0:P], start=True, stop=True,
                skip_group_check=True,
            )
        for wi in range(4):
            nc.tensor.matmul(
                warm_ps[:, 0:P], lhsT=xpair[0][:, (wi % 32) * P:(wi % 32 + 1) * P],
                rhs=ws["q"][:, 0:P], start=True, stop=True,
                skip_group_check=True,
            )

        def w_pair(nm, pidx, dtile):
            # [128, 2, 128]: kt in (2*pidx, 2*pidx+1), d-block dtile
            return ws[nm][:].rearrange("p (kt d) -> p kt d", kt=CT)[
                :, 2 * pidx:2 * pidx + 2, dtile * P:(dtile + 1) * P]

        def wfull_pair(nm, pidx):
            return ws[nm][:].rearrange("p (kt d) -> p kt d", kt=CT)[
                :, 2 * pidx:2 * pidx + 2, :]

        def x_pair(pidx, lo, size):
            return xpair[pidx][:].rearrange("p (j n) -> p j n", j=2)[:, :, lo:lo + size]

        cp_flip = 0

        def copy_rr(dst, src):
            nonlocal cp_flip
            cp_flip += 1
            if cp_flip % 2 == 0:
                nc.scalar.copy(dst, src)
            else:
                nc.vector.tensor_copy(dst, src)

        # qT[dtile]: [128, 1024] bf16; ScalarE copy adds the q bias.
        # pidx-outer so each DoubleRow weight load covers both 512-chunks.
        qT = []
        for dtile in range(CT):
            q = qpool.tile([P, NQ], dt.bfloat16, name=f"qT{dtile}")
            ps = psB.tile([P, NQ], dt.float32, name=f"qps{dtile}",
                          tag="gps", bufs=4)
            for pidx in range(2):
                for nch in range(2):
                    nc.tensor.matmul(
                        ps[:, nch * 512:(nch + 1) * 512],
                        lhsT=w_pair("q", pidx, dtile),
                        rhs=x_pair(pidx, nch * 512, 512),
                        start=(pidx == 0), stop=(pidx == 1), perf_mode=DR,
                    )
            if dtile % 2 == 0:
                nc.scalar.activation(
                    q[:], ps[:], ACTF.Identity, bias=qb4[:, dtile:dtile + 1],
                )
            else:
                nc.vector.tensor_scalar(
                    q[:], ps[:], qb4[:, dtile:dtile + 1], None, AOT.add,
                )
            qT.append(q)

        # kT[dtile]: [128, 4096] fp8e4 (stationary operand of QK); pidx-outer
        # in half-dtile groups of 4 chunks so weight loads amortize 4x.
        kT = []
        for dtile in range(CT):
            k = kpool.tile([P, N], dt.float8e4, name=f"kT{dtile}")
            pss = [psB.tile([P, NQ], dt.float32, name=f"kps{dtile}_{i}",
                            tag="gps", bufs=4) for i in range(4)]
            for pidx in range(2):
                for i in range(4):
                    for c in range(2):
                        nch = i * 2 + c
                        nc.tensor.matmul(
                            pss[i][:, c * 512:(c + 1) * 512],
                            lhsT=w_pair("k", pidx, dtile),
                            rhs=x_pair(pidx, nch * 512, 512),
                            start=(pidx == 0), stop=(pidx == 1), perf_mode=DR,
                        )
            for i in range(4):
                copy_rr(k[:, i * NQ:(i + 1) * NQ], pss[i][:])
            kT.append(k)

        # v[mt]: [128 (m), 512 (d over all heads)]
        vt2 = []
        for j in range(MT // 2):
            v = vpool.tile([P, 2 * C], dt.bfloat16, name=f"v{j}")
            ps = psB.tile([P, NQ], dt.float32, name=f"vps{j}", tag="gps", bufs=4)
            for sub in range(2):
                mt = 2 * j + sub
                for pidx in range(2):
                    nc.tensor.matmul(
                        ps[:, sub * 512:(sub + 1) * 512],
                        lhsT=x_pair(pidx, mt * P, P),
                        rhs=wfull_pair("v", pidx),
                        start=(pidx == 0), stop=(pidx == 1), perf_mode=DR,
                    )
            copy_rr(v[:], ps[:])
            vt2.append(v)

        # residual DMA late: it is only needed at the tail, so keep the
        # early HBM bandwidth for x and the weights.
        rsd_tiles = []
        rsd_engs = [nc.gpsimd, nc.scalar, nc.sync, nc.gpsimd]
        for ct in range(CT):
            rsd = rsdpool.tile([P, NQ], dt.float32, name=f"rsd{ct}", tag=f"rsd{ct}")
            rsd_engs[ct].dma_start(rsd[:], rsdc_d.ap()[ct * P:(ct + 1) * P, :])
            rsd_tiles.append(rsd)

        def vslice(mt, h):
            return vt2[mt // 2][:, (mt % 2) * C + h * HS:(mt % 2) * C + (h + 1) * HS]

        # ---------------- phase C: attention ----------------
        psctx.close()
        spool = ctx.enter_context(tc.tile_pool(name="spool", bufs=3, space="PSUM"))
        opool = ctx.enter_context(tc.tile_pool(name="opool", bufs=1, space="PSUM"))
        obpool = ctx.enter_context(tc.tile_pool(name="obpool", bufs=1, space="PSUM"))

        # steps: (pair, nch, mt) with mt innermost: oa/ob accumulate one
        # 512-query chunk at a time ([128,512] = 1 PSUM bank each), which
        # frees banks for 3-deep score double-buffering (spool bufs=3) so
        # the QK->exp->QK slot-recycling chain amortizes over 3 steps.
        steps = [
            (hp, mt, nch)
            for hp in range(NH // 2)
            for nch in range(NQ // 512)
            for mt in range(MT)
        ]

        oa_tiles = {}
        ob_tiles = {}
        s_tiles = {}

        def emit_qk(idx):
            hp, mt, nch = steps[idx]
            s = spool.tile([P, NQ], dt.float32, tag="s", name=f"s{idx}")
            kk = kT[hp]
            qq = qT[hp]
            nc.tensor.matmul(
                s[:, 0:512],
                lhsT=kk[0:64, mt * P:(mt + 1) * P],
                rhs=qq[0:64, nch * 512:(nch + 1) * 512],
                start=True, stop=True,
            )
            nc.tensor.matmul(
                s[:, 512:1024],
                lhsT=kk[64:128, mt * P:(mt + 1) * P],
                rhs=qq[64:128, nch * 512:(nch + 1) * 512],
                start=True, stop=True,
            )
            s_tiles[idx] = s

        e_tiles = {}
        on_all = onpool.tile([P, CT * NQ], dt.float8e4, name="on_all")
        exp_ctr = 0

        def emit_exp(idx):
            nonlocal exp_ctr
            s = s_tiles.pop(idx)
            kind = EXPSCHED[exp_ctr % len(EXPSCHED)]
            exp_ctr += 1
            if kind == "A":
                e = epool.tile([P, NQ], dt.bfloat16, name=f"e{idx}",
                               tag="e", bufs=4)
                nc.scalar.activation(e[:], s[:], ACTF.Exp, scale=0.125)
            else:
                e = epool.tile([P, NQ], dt.int16, name=f"e{idx}",
                               tag="e", bufs=4)
                nc.vector.tensor_scalar(
                    e[:], s[:], SCH_A16 * 0.125, SCH_B16, AOT.mult, AOT.add
                )
            e_tiles[idx] = e

        def as_bf16(e, half):
            sl = e[:, half * 512:(half + 1) * 512]
            return sl if e.tensor.dtype == dt.bfloat16 else sl.bitcast(dt.bfloat16)

        def emit_pv(idx):
            hp, mt, nch = steps[idx]
            e = e_tiles[idx]
            oa = oa_tiles[(hp, nch)]
            h0, h1 = 2 * hp, 2 * hp + 1
            first = mt == 0
            last = mt == MT - 1
            nc.tensor.matmul(
                oa[0:64, :], lhsT=vslice(mt, h0),
                rhs=as_bf16(e, 0), start=first, stop=last, skip_group_check=True,
            )
            nc.tensor.matmul(
                oa[64:128, :], lhsT=vslice(mt, h1),
                rhs=as_bf16(e, 1), start=first, stop=last,
                skip_group_check=True,
            )

        def emit_pv_ones(idx):
            hp, mt, nch = steps[idx]
            e = e_tiles.pop(idx)
            ob = ob_tiles[(hp, nch)]
            first = mt == 0
            last = mt == MT - 1
            nc.tensor.matmul(
                ob[0:64, :], lhsT=ones64[:], rhs=as_bf16(e, 0),
                start=first, stop=last, skip_group_check=True,
            )
            nc.tensor.matmul(
                ob[64:128, :], lhsT=ones64[:], rhs=as_bf16(e, 1),
                start=first, stop=last, skip_group_check=True,
            )
            if last:
                # normalize this 512-query chunk: o_norm = oA * recip_fast(oB)
                oa = oa_tiles[(hp, nch)]
                r = rpool.tile([P, 512], dt.float32, name=f"r{hp}_{nch}", tag="r")
                nc.vector.reciprocal_approx_fast(r[:], ob[:])
                nc.vector.tensor_mul(
                    on_all[:, hp * NQ + nch * 512:hp * NQ + (nch + 1) * 512],
                    oa[:], r[:])
                del oa_tiles[(hp, nch)], ob_tiles[(hp, nch)]

        NSS = len(steps) // 2
        emit_qk(0)
        emit_qk(1)
        for ss in range(NSS + 1):
            if ss < NSS:
                for st in (2 * ss, 2 * ss + 1):
                    hp, mt, nch = steps[st]
                    if mt == 0 and (hp, nch) not in oa_tiles:
                        oa_tiles[(hp, nch)] = opool.tile(
                            [P, 512], dt.float32, tag="oa", name=f"oa{hp}_{nch}")
                        ob_tiles[(hp, nch)] = obpool.tile(
                            [P, 512], dt.float32, tag="ob", name=f"ob{hp}_{nch}")
                emit_exp(2 * ss)
                emit_exp(2 * ss + 1)
            if ss > 0:
                emit_pv(2 * (ss - 1))
                emit_pv(2 * (ss - 1) + 1)
                emit_pv_ones(2 * (ss - 1))
                emit_pv_ones(2 * (ss - 1) + 1)
            if 2 * (ss + 1) < len(steps):
                emit_qk(2 * (ss + 1))
            if 2 * (ss + 1) + 1 < len(steps):
                emit_qk(2 * (ss + 1) + 1)

        # ---------------- phase D: proj + residual + out ----------------
        on_r = on_all[:].rearrange("p (kt q) -> p kt q", kt=CT)
        for ct in range(CT):
            # reuse the s-slots (freed as the tail pairs finish) so proj
            # overlaps the last accumulation group
            ys = spool.tile([P, NQ], dt.float32, tag="s", name=f"yps{ct}")
            for nch in range(NQ // 512):
                for pidx in range(2):
                    nc.tensor.matmul(
                        ys[:, nch * 512:(nch + 1) * 512],
                        lhsT=w_pair("p", pidx, ct),
                        rhs=on_r[:, 2 * pidx:2 * pidx + 2,
                                 nch * 512:(nch + 1) * 512],
                        start=(pidx == 0), stop=(pidx == 1), perf_mode=DR,
                    )
            ot = outpool.tile([P, NQ], dt.float32, name=f"ot{ct}", tag="ot")
            # y + (cout + resid), DMA each half as soon as it's done
            nc.vector.tensor_add(ot[:], ys[:], rsd_tiles[ct][:])
            for nch in range(NQ // 512):
                half = slice(nch * 512, (nch + 1) * 512)
                deng = nc.sync if (2 * ct + nch) % 2 == 0 else nc.scalar
                deng.dma_start(y_d.ap()[ct * P:(ct + 1) * P, half], ot[:, half])

    nc.compile()
    return nc


def make_in_maps(inputs):
    x = np.asarray(inputs["x"], dtype=np.float32).reshape(2, C, N)
    gn_w = np.asarray(inputs["gn_w"], np.float32)
    gn_b = np.asarray(inputs["gn_b"], np.float32)
    wq_w = np.asarray(inputs["wq_w"], np.float32)
    wk_w = np.asarray(inputs["wk_w"], np.float32)
    wv_w = np.asarray(inputs["wv_w"], np.float32)
    wp_w = np.asarray(inputs["proj_w"], np.float32)
    wq_b = np.asarray(inputs["wq_b"], np.float32)
    wv_b = np.asarray(inputs["wv_b"], np.float32)
    pj_b = np.asarray(inputs["proj_b"], np.float32)

    def t4(v):
        return np.ascontiguousarray(np.asarray(v, np.float32).reshape(CT, P).T)

    G = 32
    wpT = np.ascontiguousarray(wp_w.T).astype(FP8)
    per_batch = []
    for b in range(2):
        xg = x[b].reshape(G, C // G * N)
        mu = xg.mean(axis=1)
        var = xg.var(axis=1)
        a = gn_w * np.repeat(1.0 / np.sqrt(var + EPS), C // G)
        bb = gn_b - np.repeat(mu, C // G) * a
        wsq = np.ascontiguousarray(a[:, None] * wq_w.T).astype(FP8)
        wsk = np.ascontiguousarray(a[:, None] * wk_w.T).astype(FP8)
        wsv = np.ascontiguousarray(a[:, None] * wv_w.T).astype(FP8)
        qbias = wq_w @ bb + wq_b
        vb = wv_w @ bb + wv_b
        co = wp_w @ vb + pj_b
        per_batch.append(dict(
            wsq=wsq, wsk=wsk, wsv=wsv, wpT=wpT,
            qb4=t4(qbias), cout=co,
        ))

    in_maps = []
    for core in range(8):
        b, r = core // 4, core % 4
        nq0 = r * NQ
        rolled = np.roll(x[b], -nq0, axis=1)
        m = dict(per_batch[b])
        co = m.pop("cout")
        m["xb"] = rolled.astype(FP8)
        # residual + cout pre-added on the host
        m["rsdc"] = np.ascontiguousarray(x[b][:, nq0:nq0 + NQ] + co[:, None])
        in_maps.append(m)
    return in_maps


def assemble(results):
    out = np.empty((2, C, N), np.float32)
    for core in range(8):
        b, r = core // 4, core % 4
        out[b][:, r * NQ:(r + 1) * NQ] = results[core]["y"]
    return out.reshape(2, C, 64, 64)


def get_program():
    if "nc" not in _CACHE:
        _CACHE["nc"] = build_program()
    return _CACHE["nc"]


def kernel(**inputs):
    nc = get_program()
    in_maps = make_in_maps(inputs)
    res = run_bass_kernel_spmd(nc, in_maps, core_ids=list(range(8)))
    return assemble(res.results)


# revision 18
# speedup vs baseline: 1.0319x; 1.0002x over previous
"""Trainium2 Bass kernel for nn_AttentionBlock (GroupNorm + 8-head self-attention + proj + residual).

Full inputs in, full output out. Sharding: 8 cores = 2 batches x 4-way split of
the 4096 query pixels. Each core runs an identical SPMD program on per-core
input data (x rolled so its 1024 query pixels sit first; attention and
groupnorm are permutation-invariant over keys/pixels, so rotation is exact).

Host-side folding (exact, fp32): groupnorm h = a(.)x + b folds into the QKV
weights: ws_q = diag(a) Wq^T (fp8) with qbias = Wq b + wq_b; k's constant
cancels in softmax; v's constant rides through (cout = proj_w (Wv b + wv_b) +
proj_b added at the end). The residual+cout constant is pre-added on the host
into the rsdc tensor.

Device-side:
  QKV GEMMs in fp8 DoubleRow, pidx-outer so each stationary weight load
  covers several matmuls.  Attention: S^T tiles (keys on partitions); QK is
  emitted as four 64x64-quadrant matmuls (explicit tile_position) so all four
  run concurrently in the PE array; PV and the ones (denominator) matmuls are
  column-tiled pairs.  exp is round-robined over ScalarE (exact, 1/8 scale
  fused), Pool/GpSimd and DVE (int16-Schraudolph); o_norm = oA * recip(oB);
  y^T = proj_w o_norm^T + rsdc, DMA'd out per 512-chunk.
"""

import numpy as np
import ml_dtypes
from contextlib import ExitStack

import concourse.bacc as bacc
import concourse.tile as tile
import concourse.mybir as mybir
from concourse.bass_utils import run_bass_kernel_spmd

BF16 = ml_dtypes.bfloat16
FP8 = ml_dtypes.float8_e4m3
F32 = np.float32

P = 128          # partitions
C = 512          # channels
NH = 8
HS = 64
N = 4096         # pixels (keys)
NQ = 1024        # queries per core
CT = 4           # channel tiles of 128
MT = 32          # m (key) tiles of 128
EPS = 1e-5
SCH_A16 = float(2.0 ** 7 / np.log(2.0))   # int16 Schraudolph exp
SCH_B16 = 16250.4062

dt = mybir.dt
AOT = mybir.AluOpType
ACTF = mybir.ActivationFunctionType
AXT = mybir.AxisListType

_CACHE = {}

# exp engine schedule per score tile: P=pool/gpsimd, A=scalar-exact, D=dve
EXPSCHED = "AD"


def build_program():
    nc = bacc.Bacc("TRN2", target_bir_lowering=False, debug=False, num_devices=8)

    xb_d = nc.dram_tensor("xb", [C, N], dt.float8e4, kind="ExternalInput")
    rsdc_d = nc.dram_tensor("rsdc", [C, NQ], dt.bfloat16, kind="ExternalInput")
    wsq_d = nc.dram_tensor("wsq", [C, C], dt.float8e4, kind="ExternalInput")
    wsk_d = nc.dram_tensor("wsk", [C, C], dt.float8e4, kind="ExternalInput")
    wsv_d = nc.dram_tensor("wsv", [C, C], dt.float8e4, kind="ExternalInput")
    wp_d = nc.dram_tensor("wpT", [C, C], dt.float8e4, kind="ExternalInput")
    qb_d = nc.dram_tensor("qb4", [P, CT], dt.float32, kind="ExternalInput")
    y_d = nc.dram_tensor("y", [C, NQ], dt.float32, kind="ExternalOutput")

    with tile.TileContext(nc) as tc, ExitStack() as ctx:
        const = ctx.enter_context(tc.tile_pool(name="const", bufs=1))
        wpool = ctx.enter_context(tc.tile_pool(name="wpool", bufs=1))
        xpool = ctx.enter_context(tc.tile_pool(name="xpool", bufs=1))
        kpool = ctx.enter_context(tc.tile_pool(name="kpool", bufs=1))
        qpool = ctx.enter_context(tc.tile_pool(name="qpool", bufs=1))
        vpool = ctx.enter_context(tc.tile_pool(name="vpool", bufs=1))
        epool = ctx.enter_context(tc.tile_pool(name="epool", bufs=3))
        onpool = ctx.enter_context(tc.tile_pool(name="onpool", bufs=1))
        rpool = ctx.enter_context(tc.tile_pool(name="rpool", bufs=2))
        rsdpool = ctx.enter_context(tc.tile_pool(name="rsdpool", bufs=2))
        outpool = ctx.enter_context(tc.tile_pool(name="outpool", bufs=4))

        # ---------------- input DMA ----------------
        # tiny fp8 q/k weights first (they gate the first GEMMs), then the
        # 2MB xb split over four queues; wsv/qb next (gate V/q-bias); wp and
        # the residual go last (needed only at the tail).
        rr = [nc.sync, nc.scalar, nc.gpsimd]
        ws = {}
        w = wpool.tile([P, CT * C], dt.float8e4, tag="ws_q", name="ws_q")
        # chunk 0 first: it alone gates the HAM warm-up matmuls
        nc.sync.dma_start(w[:, 0:C], wsq_d.ap()[0:P, :])
        ws["q"] = w

        # all of x next, as 8 half-tiles round-robined over the 3 rings so
        # the QKV gemms are never starved
        xpair = [xpool.tile([P, 2 * N], dt.float8e4, name=f"xp{pidx}")
                 for pidx in range(2)]
        for i in range(8):
            t, u = i // 2, i % 2
            rr[i % 3].dma_start(
                xpair[t // 2][:, (t % 2) * N + u * 2048:(t % 2) * N + (u + 1) * 2048],
                xb_d.ap()[t * P:(t + 1) * P, u * 2048:(u + 1) * 2048],
            )
        for kt in range(1, CT):
            nc.sync.dma_start(
                w[:, kt * C:(kt + 1) * C], wsq_d.ap()[kt * P:(kt + 1) * P, :]
            )

        w = wpool.tile([P, CT * C], dt.float8e4, tag="ws_k", name="ws_k")
        for kt in range(CT):
            nc.scalar.dma_start(
                w[:, kt * C:(kt + 1) * C], wsk_d.ap()[kt * P:(kt + 1) * P, :]
            )
        ws["k"] = w
        qb4 = const.tile([P, CT], dt.float32)
        nc.sync.dma_start(qb4[:], qb_d.ap())
        wsv = wpool.tile([P, CT * C], dt.float8e4, tag="ws_v", name="ws_v")
        for kt in range(CT):
            nc.gpsimd.dma_start(
                wsv[:, kt * C:(kt + 1) * C], wsv_d.ap()[kt * P:(kt + 1) * P, :]
            )
        ws["v"] = wsv
        wp = wpool.tile([P, CT * C], dt.float8e4, tag="w_p", name="w_p")
        for kt in range(CT):
            eng = nc.scalar if kt % 2 == 0 else nc.gpsimd
            eng.dma_start(
                wp[:, kt * C:(kt + 1) * C], wp_d.ap()[kt * P:(kt + 1) * P, :]
            )
        ws["p"] = wp
        ones64 = const.tile([P, HS], dt.bfloat16)
        nc.vector.memset(ones64[:], 1.0)

        # ---------------- phase B: QKV GEMMs (fp8 DoubleRow) ----------------
        psctx = ExitStack()
        psB = psctx.enter_context(tc.tile_pool(name="psB", bufs=1, space="PSUM"))

        DR = mybir.MatmulPerfMode.DoubleRow

        # HAM warm-up: the PE idles during the input DMA (~10-20us), so the
        # first QKV matmuls would run at the cold K=4/8 clock for ~3.4us.
        # Keep the PE "busy" with tiny matmuls gated only on the earliest
        # arrivals (ws_q, then xb tile 0) so the clock gate opens before the
        # real GEMMs start.
        warm_ps = psB.tile([P, NQ], dt.float32, tag="gps", name="warm_ps", bufs=4)
        for wi in range(8):
            nc.tensor.matmul(
                warm_ps[:, 0:P], lhsT=ws["q"][:, (wi % 4) * P:(wi % 4 + 1) * P],
                rhs=ws["q"][:, 0:P], start=True, stop=True,
                skip_group_check=True,
            )
        for wi in range(4):
            nc.tensor.matmul(
                warm_ps[:, 0:P], lhsT=xpair[0][:, (wi % 32) * P:(wi % 32 + 1) * P],
                rhs=ws["q"][:, 0:P], start=True, stop=True,
                skip_group_check=True,
            )

        def w_pair(nm, pidx, dtile):
            # [128, 2, 128]: kt in (2*pidx, 2*pidx+1), d-block dtile
            return ws[nm][:].rearrange("p (kt d) -> p kt d", kt=CT)[
                :, 2 * pidx:2 * pidx + 2, dtile * P:(dtile + 1) * P]

        def wfull_pair(nm, pidx):
            return ws[nm][:].rearrange("p (kt d) -> p kt d", kt=CT)[
                :, 2 * pidx:2 * pidx + 2, :]

        def x_pair(pidx, lo, size):
            return xpair[pidx][:].rearrange("p (j n) -> p j n", j=2)[:, :, lo:lo + size]

        cp_flip = 0

        def copy_rr(dst, src):
            nonlocal cp_flip
            cp_flip += 1
            if cp_flip % 2 == 0:
                nc.scalar.copy(dst, src)
            else:
                nc.vector.tensor_copy(dst, src)

        # qT[dtile]: [128, 1024] bf16; ScalarE copy adds the q bias.
        # pidx-outer so each DoubleRow weight load covers both 512-chunks.
        qT = []
        for dtile in range(CT):
            q = qpool.tile([P, NQ], dt.bfloat16, name=f"qT{dtile}")
            ps = psB.tile([P, NQ], dt.float32, name=f"qps{dtile}",
                          tag="gps", bufs=4)
            for pidx in range(2):
                for nch in range(2):
                    nc.tensor.matmul(
                        ps[:, nch * 512:(nch + 1) * 512],
                        lhsT=w_pair("q", pidx, dtile),
                        rhs=x_pair(pidx, nch * 512, 512),
                        start=(pidx == 0), stop=(pidx == 1), perf_mode=DR,
                    )
            if dtile % 2 == 0:
                nc.scalar.activation(
                    q[:], ps[:], ACTF.Identity, bias=qb4[:, dtile:dtile + 1],
                )
            else:
                nc.vector.tensor_scalar(
                    q[:], ps[:], qb4[:, dtile:dtile + 1], None, AOT.add,
                )
            qT.append(q)

        # kT[dtile]: [128, 4096] fp8e4 (stationary operand of QK); pidx-outer
        # in half-dtile groups of 4 chunks so weight loads amortize 4x.
        kT = []
        for dtile in range(CT):
            k = kpool.tile([P, N], dt.float8e4, name=f"kT{dtile}")
            pss = [psB.tile([P, NQ], dt.float32, name=f"kps{dtile}_{i}",
                            tag="gps", bufs=4) for i in range(4)]
            for pidx in range(2):
                for i in range(4):
                    for c in range(2):
                        nch = i * 2 + c
                        nc.tensor.matmul(
                            pss[i][:, c * 512:(c + 1) * 512],
                            lhsT=w_pair("k", pidx, dtile),
                            rhs=x_pair(pidx, nch * 512, 512),
                            start=(pidx == 0), stop=(pidx == 1), perf_mode=DR,
                        )
            for i in range(4):
                copy_rr(k[:, i * NQ:(i + 1) * NQ], pss[i][:])
            kT.append(k)

        # v[mt]: [128 (m), 512 (d over all heads)]
        vt2 = []
        for j in range(MT // 2):
            v = vpool.tile([P, 2 * C], dt.bfloat16, name=f"v{j}")
            ps = psB.tile([P, NQ], dt.float32, name=f"vps{j}", tag="gps", bufs=4)
            for sub in range(2):
                mt = 2 * j + sub
                for pidx in range(2):
                    nc.tensor.matmul(
                        ps[:, sub * 512:(sub + 1) * 512],
                        lhsT=x_pair(pidx, mt * P, P),
                        rhs=wfull_pair("v", pidx),
                        start=(pidx == 0), stop=(pidx == 1), perf_mode=DR,
                    )
            copy_rr(v[:], ps[:])
            vt2.append(v)

        # residual DMA late: it is only needed at the tail, so keep the
        # early HBM bandwidth for x and the weights.
        rsd_tiles = []
        rsd_engs = [nc.gpsimd, nc.scalar, nc.sync, nc.gpsimd]
        for ct in range(CT):
            rsd = rsdpool.tile([P, NQ], dt.bfloat16, name=f"rsd{ct}", tag=f"rsd{ct}")
            rsd_engs[ct].dma_start(rsd[:], rsdc_d.ap()[ct * P:(ct + 1) * P, :])
            rsd_tiles.append(rsd)

        def vslice(mt, h):
            return vt2[mt // 2][:, (mt % 2) * C + h * HS:(mt % 2) * C + (h + 1) * HS]

        # ---------------- phase C: attention ----------------
        psctx.close()
        spool = ctx.enter_context(tc.tile_pool(name="spool", bufs=3, space="PSUM"))
        opool = ctx.enter_context(tc.tile_pool(name="opool", bufs=1, space="PSUM"))
        obpool = ctx.enter_context(tc.tile_pool(name="obpool", bufs=1, space="PSUM"))

        # steps: (pair, nch, mt) with mt innermost: oa/ob accumulate one
        # 512-query chunk at a time ([128,512] = 1 PSUM bank each), which
        # frees banks for 3-deep score double-buffering (spool bufs=3) so
        # the QK->exp->QK slot-recycling chain amortizes over 3 steps.
        steps = [
            (hp, mt, nch)
            for hp in range(NH // 2)
            for nch in range(NQ // 512)
            for mt in range(MT)
        ]

        oa_tiles = {}
        ob_tiles = {}
        s_tiles = {}

        def emit_qk(idx):
            hp, mt, nch = steps[idx]
            s = spool.tile([P, NQ], dt.float32, tag="s", name=f"s{idx}")
            kk = kT[hp]
            qq = qT[hp]
            nc.tensor.matmul(
                s[:, 0:512],
                lhsT=kk[0:64, mt * P:(mt + 1) * P],
                rhs=qq[0:64, nch * 512:(nch + 1) * 512],
                start=True, stop=True,
            )
            nc.tensor.matmul(
                s[:, 512:1024],
                lhsT=kk[64:128, mt * P:(mt + 1) * P],
                rhs=qq[64:128, nch * 512:(nch + 1) * 512],
                start=True, stop=True,
            )
            s_tiles[idx] = s

        e_tiles = {}
        on_all = onpool.tile([P, CT * NQ], dt.float8e4, name="on_all")
        exp_ctr = 0

        def emit_exp(idx):
            nonlocal exp_ctr
            s = s_tiles.pop(idx)
            kind = EXPSCHED[exp_ctr % len(EXPSCHED)]
            exp_ctr += 1
            if kind == "A":
                e = epool.tile([P, NQ], dt.bfloat16, name=f"e{idx}",
                               tag="e", bufs=4)
                nc.scalar.activation(e[:], s[:], ACTF.Exp, scale=0.125)
            else:
                e = epool.tile([P, NQ], dt.int16, name=f"e{idx}",
                               tag="e", bufs=4)
                nc.vector.tensor_scalar(
                    e[:], s[:], SCH_A16 * 0.125, SCH_B16, AOT.mult, AOT.add
                )
            e_tiles[idx] = e

        def as_bf16(e, half):
            sl = e[:, half * 512:(half + 1) * 512]
            return sl if e.tensor.dtype == dt.bfloat16 else sl.bitcast(dt.bfloat16)

        def emit_pv(idx):
            hp, mt, nch = steps[idx]
            e = e_tiles[idx]
            oa = oa_tiles[(hp, nch)]
            h0, h1 = 2 * hp, 2 * hp + 1
            first = mt == 0
            last = mt == MT - 1
            nc.tensor.matmul(
                oa[0:64, :], lhsT=vslice(mt, h0),
                rhs=as_bf16(e, 0), start=first, stop=last, skip_group_check=True,
            )
            nc.tensor.matmul(
                oa[64:128, :], lhsT=vslice(mt, h1),
                rhs=as_bf16(e, 1), start=first, stop=last,
                skip_group_check=True,
            )

        def emit_pv_ones(idx):
            hp, mt, nch = steps[idx]
            e = e_tiles.pop(idx)
            ob = ob_tiles[(hp, nch)]
            first = mt == 0
            last = mt == MT - 1
            nc.tensor.matmul(
                ob[0:64, :], lhsT=ones64[:], rhs=as_bf16(e, 0),
                start=first, stop=last, skip_group_check=True,
            )
            nc.tensor.matmul(
                ob[64:128, :], lhsT=ones64[:], rhs=as_bf16(e, 1),
                start=first, stop=last, skip_group_check=True,
            )
            if last:
                # normalize this 512-query chunk: o_norm = oA * recip_fast(oB)
                oa = oa_tiles[(hp, nch)]
                r = rpool.tile([P, 512], dt.float32, name=f"r{hp}_{nch}", tag="r")
                nc.vector.reciprocal_approx_fast(r[:], ob[:])
                nc.vector.tensor_mul(
                    on_all[:, hp * NQ + nch * 512:hp * NQ + (nch + 1) * 512],
                    oa[:], r[:])
                del oa_tiles[(hp, nch)], ob_tiles[(hp, nch)]

        NSS = len(steps) // 2
        emit_qk(0)
        emit_qk(1)
        for ss in range(NSS + 1):
            if ss < NSS:
                for st in (2 * ss, 2 * ss + 1):
                    hp, mt, nch = steps[st]
                    if mt == 0 and (hp, nch) not in oa_tiles:
                        oa_tiles[(hp, nch)] = opool.tile(
                            [P, 512], dt.float32, tag="oa", name=f"oa{hp}_{nch}")
                        ob_tiles[(hp, nch)] = obpool.tile(
                            [P, 512], dt.float32, tag="ob", name=f"ob{hp}_{nch}")
                emit_exp(2 * ss)
                emit_exp(2 * ss + 1)
            if ss > 0:
                emit_pv(2 * (ss - 1))
                emit_pv(2 * (ss - 1) + 1)
                emit_pv_ones(2 * (ss - 1))
                emit_pv_ones(2 * (ss - 1) + 1)
            if 2 * (ss + 1) < len(steps):
                emit_qk(2 * (ss + 1))
            if 2 * (ss + 1) + 1 < len(steps):
                emit_qk(2 * (ss + 1) + 1)

        # ---------------- phase D: proj + residual + out ----------------
        on_r = on_all[:].rearrange("p (kt q) -> p kt q", kt=CT)
        for ct in range(CT):
            # reuse the s-slots (freed as the tail pairs finish) so proj
            # overlaps the last accumulation group
            ys = spool.tile([P, NQ], dt.float32, tag="s", name=f"yps{ct}")
            for nch in range(NQ // 512):
                for pidx in range(2):
                    nc.tensor.matmul(
                        ys[:, nch * 512:(nch + 1) * 512],
                        lhsT=w_pair("p", pidx, ct),
                        rhs=on_r[:, 2 * pidx:2 * pidx + 2,
                                 nch * 512:(nch + 1) * 512],
                        start=(pidx == 0), stop=(pidx == 1), perf_mode=DR,
                    )
            ot = outpool.tile([P, NQ], dt.float32, name=f"ot{ct}", tag="ot")
            # y + (cout + resid), DMA each half as soon as it's done
            nc.vector.tensor_add(ot[:], ys[:], rsd_tiles[ct][:])
            for nch in range(NQ // 512):
                half = slice(nch * 512, (nch + 1) * 512)
                deng = nc.sync if (2 * ct + nch) % 2 == 0 else nc.scalar
                deng.dma_start(y_d.ap()[ct * P:(ct + 1) * P, half], ot[:, half])

    nc.compile()
    return nc


def make_in_maps(inputs):
    x = np.asarray(inputs["x"], dtype=np.float32).reshape(2, C, N)
    gn_w = np.asarray(inputs["gn_w"], np.float32)
    gn_b = np.asarray(inputs["gn_b"], np.float32)
    wq_w = np.asarray(inputs["wq_w"], np.float32)
    wk_w = np.asarray(inputs["wk_w"], np.float32)
    wv_w = np.asarray(inputs["wv_w"], np.float32)
    wp_w = np.asarray(inputs["proj_w"], np.float32)
    wq_b = np.asarray(inputs["wq_b"], np.float32)
    wv_b = np.asarray(inputs["wv_b"], np.float32)
    pj_b = np.asarray(inputs["proj_b"], np.float32)

    def t4(v):
        return np.ascontiguousarray(np.asarray(v, np.float32).reshape(CT, P).T)

    G = 32
    wpT = np.ascontiguousarray(wp_w.T).astype(FP8)
    per_batch = []
    for b in range(2):
        xg = x[b].reshape(G, C // G * N)
        mu = xg.mean(axis=1)
        var = xg.var(axis=1)
        a = gn_w * np.repeat(1.0 / np.sqrt(var + EPS), C // G)
        bb = gn_b - np.repeat(mu, C // G) * a
        wsq = np.ascontiguousarray(a[:, None] * wq_w.T).astype(FP8)
        wsk = np.ascontiguousarray(a[:, None] * wk_w.T).astype(FP8)
        wsv = np.ascontiguousarray(a[:, None] * wv_w.T).astype(FP8)
        qbias = wq_w @ bb + wq_b
        vb = wv_w @ bb + wv_b
        co = wp_w @ vb + pj_b
        per_batch.append(dict(
            wsq=wsq, wsk=wsk, wsv=wsv, wpT=wpT,
            qb4=t4(qbias), cout=co,
        ))

    in_maps = []
    for core in range(8):
        b, r = core // 4, core % 4
        nq0 = r * NQ
        rolled = np.roll(x[b], -nq0, axis=1)
        m = dict(per_batch[b])
        co = m.pop("cout")
        m["xb"] = rolled.astype(FP8)
        # residual + cout pre-added on the host
        m["rsdc"] = np.ascontiguousarray(x[b][:, nq0:nq0 + NQ] + co[:, None]).astype(BF16)
        in_maps.append(m)
    return in_maps


def assemble(results):
    out = np.empty((2, C, N), np.float32)
    for core in range(8):
        b, r = core // 4, core % 4
        out[b][:, r * NQ:(r + 1) * NQ] = results[core]["y"]
    return out.reshape(2, C, 64, 64)


def get_program():
    if "nc" not in _CACHE:
        _CACHE["nc"] = build_program()
    return _CACHE["nc"]


def kernel(**inputs):
    nc = get_program()
    in_maps = make_in_maps(inputs)
    res = run_bass_kernel_spmd(nc, in_maps, core_ids=list(range(8)))
    return assemble(res.results)


# revision 19
# speedup vs baseline: 1.0409x; 1.0086x over previous
"""Trainium2 Bass kernel for nn_AttentionBlock (GroupNorm + 8-head self-attention + proj + residual).

Full inputs in, full output out. Sharding: 8 cores = 2 batches x 4-way split of
the 4096 query pixels. Each core runs an identical SPMD program on per-core
input data (x rolled so its 1024 query pixels sit first; attention and
groupnorm are permutation-invariant over keys/pixels, so rotation is exact).

Host-side folding (exact, fp32): groupnorm h = a(.)x + b folds into the QKV
weights: ws_q = diag(a) Wq^T (fp8) with qbias = Wq b + wq_b; k's constant
cancels in softmax; v's constant rides through (cout = proj_w (Wv b + wv_b) +
proj_b added at the end). The residual+cout constant is pre-added on the host
into the rsdc tensor.

Device-side:
  QKV GEMMs in fp8 DoubleRow, pidx-outer so each stationary weight load
  covers several matmuls.  Attention: S^T tiles (keys on partitions); QK is
  emitted as four 64x64-quadrant matmuls (explicit tile_position) so all four
  run concurrently in the PE array; PV and the ones (denominator) matmuls are
  column-tiled pairs.  exp is round-robined over ScalarE (exact, 1/8 scale
  fused), Pool/GpSimd and DVE (int16-Schraudolph); o_norm = oA * recip(oB);
  y^T = proj_w o_norm^T + rsdc, DMA'd out per 512-chunk.
"""

import numpy as np
import ml_dtypes
from contextlib import ExitStack

import concourse.bacc as bacc
import concourse.tile as tile
import concourse.mybir as mybir
from concourse.bass_utils import run_bass_kernel_spmd

BF16 = ml_dtypes.bfloat16
FP8 = ml_dtypes.float8_e4m3
F32 = np.float32

P = 128          # partitions
C = 512          # channels
NH = 8
HS = 64
N = 4096         # pixels (keys)
NQ = 1024        # queries per core
CT = 4           # channel tiles of 128
MT = 32          # m (key) tiles of 128
EPS = 1e-5
SCH_A16 = float(2.0 ** 7 / np.log(2.0))   # int16 Schraudolph exp
SCH_B16 = 16250.4062

dt = mybir.dt
AOT = mybir.AluOpType
ACTF = mybir.ActivationFunctionType
AXT = mybir.AxisListType

_CACHE = {}

# exp engine schedule per score tile: P=pool/gpsimd, A=scalar-exact, D=dve
EXPSCHED = "AD"


def build_program():
    nc = bacc.Bacc("TRN2", target_bir_lowering=False, debug=False, num_devices=8)

    xb_d = nc.dram_tensor("xb", [C, N], dt.float8e4, kind="ExternalInput")
    rsdc_d = nc.dram_tensor("rsdc", [C, NQ], dt.bfloat16, kind="ExternalInput")
    wsq_d = nc.dram_tensor("wsq", [C, C], dt.float8e4, kind="ExternalInput")
    wsk_d = nc.dram_tensor("wsk", [C, C], dt.float8e4, kind="ExternalInput")
    wsv_d = nc.dram_tensor("wsv", [C, C], dt.float8e4, kind="ExternalInput")
    wp_d = nc.dram_tensor("wpT", [C, C], dt.float8e4, kind="ExternalInput")
    qb_d = nc.dram_tensor("qb4", [P, CT], dt.float32, kind="ExternalInput")
    y_d = nc.dram_tensor("y", [C, NQ], dt.float32, kind="ExternalOutput")

    with tile.TileContext(nc) as tc, ExitStack() as ctx:
        const = ctx.enter_context(tc.tile_pool(name="const", bufs=1))
        wpool = ctx.enter_context(tc.tile_pool(name="wpool", bufs=1))
        xpool = ctx.enter_context(tc.tile_pool(name="xpool", bufs=1))
        kpool = ctx.enter_context(tc.tile_pool(name="kpool", bufs=1))
        qpool = ctx.enter_context(tc.tile_pool(name="qpool", bufs=1))
        vpool = ctx.enter_context(tc.tile_pool(name="vpool", bufs=1))
        epool = ctx.enter_context(tc.tile_pool(name="epool", bufs=3))
        onpool = ctx.enter_context(tc.tile_pool(name="onpool", bufs=1))
        rpool = ctx.enter_context(tc.tile_pool(name="rpool", bufs=2))
        rsdpool = ctx.enter_context(tc.tile_pool(name="rsdpool", bufs=2))
        outpool = ctx.enter_context(tc.tile_pool(name="outpool", bufs=4))

        # ---------------- input DMA ----------------
        # tiny fp8 q/k weights first (they gate the first GEMMs), then the
        # 2MB xb split over four queues; wsv/qb next (gate V/q-bias); wp and
        # the residual go last (needed only at the tail).
        rr = [nc.sync, nc.scalar, nc.gpsimd]
        ws = {}
        w = wpool.tile([P, CT * C], dt.float8e4, tag="ws_q", name="ws_q")
        # chunk 0 first: it alone gates the HAM warm-up matmuls
        nc.sync.dma_start(w[:, 0:C], wsq_d.ap()[0:P, :])
        ws["q"] = w

        # all of x next, as 8 half-tiles round-robined over the 3 rings so
        # the QKV gemms are never starved
        xpair = [xpool.tile([P, 2 * N], dt.float8e4, name=f"xp{pidx}")
                 for pidx in range(2)]
        xasg = [nc.sync, nc.scalar, nc.gpsimd, nc.scalar, nc.gpsimd,
                nc.sync, nc.scalar, nc.gpsimd]
        for i in range(8):
            t, u = i // 2, i % 2
            xasg[i].dma_start(
                xpair[t // 2][:, (t % 2) * N + u * 2048:(t % 2) * N + (u + 1) * 2048],
                xb_d.ap()[t * P:(t + 1) * P, u * 2048:(u + 1) * 2048],
            )
        for kt in range(1, CT):
            nc.sync.dma_start(
                w[:, kt * C:(kt + 1) * C], wsq_d.ap()[kt * P:(kt + 1) * P, :]
            )

        w = wpool.tile([P, CT * C], dt.float8e4, tag="ws_k", name="ws_k")
        for kt in range(CT):
            nc.scalar.dma_start(
                w[:, kt * C:(kt + 1) * C], wsk_d.ap()[kt * P:(kt + 1) * P, :]
            )
        ws["k"] = w
        qb4 = const.tile([P, CT], dt.float32)
        nc.sync.dma_start(qb4[:], qb_d.ap())
        wsv = wpool.tile([P, CT * C], dt.float8e4, tag="ws_v", name="ws_v")
        for kt in range(CT):
            nc.gpsimd.dma_start(
                wsv[:, kt * C:(kt + 1) * C], wsv_d.ap()[kt * P:(kt + 1) * P, :]
            )
        ws["v"] = wsv
        wp = wpool.tile([P, CT * C], dt.float8e4, tag="w_p", name="w_p")
        for kt in range(CT):
            eng = nc.scalar if kt % 2 == 0 else nc.gpsimd
            eng.dma_start(
                wp[:, kt * C:(kt + 1) * C], wp_d.ap()[kt * P:(kt + 1) * P, :]
            )
        ws["p"] = wp
        ones64 = const.tile([P, HS], dt.bfloat16)
        nc.vector.memset(ones64[:], 1.0)

        # ---------------- phase B: QKV GEMMs (fp8 DoubleRow) ----------------
        psctx = ExitStack()
        psB = psctx.enter_context(tc.tile_pool(name="psB", bufs=1, space="PSUM"))

        DR = mybir.MatmulPerfMode.DoubleRow

        # HAM warm-up: the PE idles during the input DMA (~10-20us), so the
        # first QKV matmuls would run at the cold K=4/8 clock for ~3.4us.
        # Keep the PE "busy" with tiny matmuls gated only on the earliest
        # arrivals (ws_q, then xb tile 0) so the clock gate opens before the
        # real GEMMs start.
        warm_ps = psB.tile([P, NQ], dt.float32, tag="gps", name="warm_ps", bufs=4)
        for wi in range(8):
            nc.tensor.matmul(
                warm_ps[:, 0:P], lhsT=ws["q"][:, (wi % 4) * P:(wi % 4 + 1) * P],
                rhs=ws["q"][:, 0:P], start=True, stop=True,
                skip_group_check=True,
            )
        for wi in range(4):
            nc.tensor.matmul(
                warm_ps[:, 0:P], lhsT=xpair[0][:, (wi % 32) * P:(wi % 32 + 1) * P],
                rhs=ws["q"][:, 0:P], start=True, stop=True,
                skip_group_check=True,
            )

        def w_pair(nm, pidx, dtile):
            # [128, 2, 128]: kt in (2*pidx, 2*pidx+1), d-block dtile
            return ws[nm][:].rearrange("p (kt d) -> p kt d", kt=CT)[
                :, 2 * pidx:2 * pidx + 2, dtile * P:(dtile + 1) * P]

        def wfull_pair(nm, pidx):
            return ws[nm][:].rearrange("p (kt d) -> p kt d", kt=CT)[
                :, 2 * pidx:2 * pidx + 2, :]

        def x_pair(pidx, lo, size):
            return xpair[pidx][:].rearrange("p (j n) -> p j n", j=2)[:, :, lo:lo + size]

        cp_flip = 0

        def copy_rr(dst, src):
            nonlocal cp_flip
            cp_flip += 1
            if cp_flip % 2 == 0:
                nc.scalar.copy(dst, src)
            else:
                nc.vector.tensor_copy(dst, src)

        # qT[dtile]: [128, 1024] bf16; ScalarE copy adds the q bias.
        # pidx-outer so each DoubleRow weight load covers both 512-chunks.
        qT = []
        for dtile in range(CT):
            q = qpool.tile([P, NQ], dt.bfloat16, name=f"qT{dtile}")
            ps = psB.tile([P, NQ], dt.float32, name=f"qps{dtile}",
                          tag="gps", bufs=4)
            for pidx in range(2):
                for nch in range(2):
                    nc.tensor.matmul(
                        ps[:, nch * 512:(nch + 1) * 512],
                        lhsT=w_pair("q", pidx, dtile),
                        rhs=x_pair(pidx, nch * 512, 512),
                        start=(pidx == 0), stop=(pidx == 1), perf_mode=DR,
                    )
            if dtile % 2 == 0:
                nc.scalar.activation(
                    q[:], ps[:], ACTF.Identity, bias=qb4[:, dtile:dtile + 1],
                )
            else:
                nc.vector.tensor_scalar(
                    q[:], ps[:], qb4[:, dtile:dtile + 1], None, AOT.add,
                )
            qT.append(q)

        # kT[dtile]: [128, 4096] fp8e4 (stationary operand of QK); pidx-outer
        # in half-dtile groups of 4 chunks so weight loads amortize 4x.
        kT = []
        for dtile in range(CT):
            k = kpool.tile([P, N], dt.float8e4, name=f"kT{dtile}")
            for grp in range(4):
                ps = psB.tile([P, NQ], dt.float32, name=f"kps{dtile}_{grp}",
                              tag="gps", bufs=4)
                for pidx in range(2):
                    for c in range(2):
                        nch = grp * 2 + c
                        nc.tensor.matmul(
                            ps[:, c * 512:(c + 1) * 512],
                            lhsT=w_pair("k", pidx, dtile),
                            rhs=x_pair(pidx, nch * 512, 512),
                            start=(pidx == 0), stop=(pidx == 1), perf_mode=DR,
                        )
                copy_rr(k[:, grp * NQ:(grp + 1) * NQ], ps[:])
            kT.append(k)

        # v[mt]: [128 (m), 512 (d over all heads)]
        vt2 = []
        for j in range(MT // 2):
            v = vpool.tile([P, 2 * C], dt.bfloat16, name=f"v{j}")
            ps = psB.tile([P, NQ], dt.float32, name=f"vps{j}", tag="gps", bufs=4)
            for sub in range(2):
                mt = 2 * j + sub
                for pidx in range(2):
                    nc.tensor.matmul(
                        ps[:, sub * 512:(sub + 1) * 512],
                        lhsT=x_pair(pidx, mt * P, P),
                        rhs=wfull_pair("v", pidx),
                        start=(pidx == 0), stop=(pidx == 1), perf_mode=DR,
                    )
            copy_rr(v[:], ps[:])
            vt2.append(v)

        # residual DMA late: it is only needed at the tail, so keep the
        # early HBM bandwidth for x and the weights.
        rsd_tiles = []
        rsd_engs = [nc.gpsimd, nc.scalar, nc.sync, nc.gpsimd]
        for ct in range(CT):
            rsd = rsdpool.tile([P, NQ], dt.bfloat16, name=f"rsd{ct}", tag=f"rsd{ct}")
            rsd_engs[ct].dma_start(rsd[:], rsdc_d.ap()[ct * P:(ct + 1) * P, :])
            rsd_tiles.append(rsd)

        def vslice(mt, h):
            return vt2[mt // 2][:, (mt % 2) * C + h * HS:(mt % 2) * C + (h + 1) * HS]

        # ---------------- phase C: attention ----------------
        psctx.close()
        spool = ctx.enter_context(tc.tile_pool(name="spool", bufs=3, space="PSUM"))
        opool = ctx.enter_context(tc.tile_pool(name="opool", bufs=1, space="PSUM"))
        obpool = ctx.enter_context(tc.tile_pool(name="obpool", bufs=1, space="PSUM"))

        # steps: (pair, nch, mt) with mt innermost: oa/ob accumulate one
        # 512-query chunk at a time ([128,512] = 1 PSUM bank each), which
        # frees banks for 3-deep score double-buffering (spool bufs=3) so
        # the QK->exp->QK slot-recycling chain amortizes over 3 steps.
        steps = [
            (hp, mt, nch)
            for hp in range(NH // 2)
            for nch in range(NQ // 512)
            for mt in range(MT)
        ]

        oa_tiles = {}
        ob_tiles = {}
        s_tiles = {}

        def emit_qk(idx):
            hp, mt, nch = steps[idx]
            s = spool.tile([P, NQ], dt.float32, tag="s", name=f"s{idx}")
            kk = kT[hp]
            qq = qT[hp]
            nc.tensor.matmul(
                s[:, 0:512],
                lhsT=kk[0:64, mt * P:(mt + 1) * P],
                rhs=qq[0:64, nch * 512:(nch + 1) * 512],
                start=True, stop=True,
            )
            nc.tensor.matmul(
                s[:, 512:1024],
                lhsT=kk[64:128, mt * P:(mt + 1) * P],
                rhs=qq[64:128, nch * 512:(nch + 1) * 512],
                start=True, stop=True,
            )
            s_tiles[idx] = s

        e_tiles = {}
        on_all = onpool.tile([P, CT * NQ], dt.float8e4, name="on_all")
        exp_ctr = 0

        def emit_exp(idx):
            nonlocal exp_ctr
            s = s_tiles.pop(idx)
            kind = EXPSCHED[exp_ctr % len(EXPSCHED)]
            exp_ctr += 1
            if kind == "A":
                e = epool.tile([P, NQ], dt.bfloat16, name=f"e{idx}",
                               tag="e", bufs=4)
                nc.scalar.activation(e[:], s[:], ACTF.Exp, scale=0.125)
            else:
                e = epool.tile([P, NQ], dt.int16, name=f"e{idx}",
                               tag="e", bufs=4)
                nc.vector.tensor_scalar(
                    e[:], s[:], SCH_A16 * 0.125, SCH_B16, AOT.mult, AOT.add
                )
            e_tiles[idx] = e

        def as_bf16(e, half):
            sl = e[:, half * 512:(half + 1) * 512]
            return sl if e.tensor.dtype == dt.bfloat16 else sl.bitcast(dt.bfloat16)

        def emit_pv(idx):
            hp, mt, nch = steps[idx]
            e = e_tiles[idx]
            oa = oa_tiles[(hp, nch)]
            h0, h1 = 2 * hp, 2 * hp + 1
            first = mt == 0
            last = mt == MT - 1
            nc.tensor.matmul(
                oa[0:64, :], lhsT=vslice(mt, h0),
                rhs=as_bf16(e, 0), start=first, stop=last, skip_group_check=True,
            )
            nc.tensor.matmul(
                oa[64:128, :], lhsT=vslice(mt, h1),
                rhs=as_bf16(e, 1), start=first, stop=last,
                skip_group_check=True,
            )

        def emit_pv_ones(idx):
            hp, mt, nch = steps[idx]
            e = e_tiles.pop(idx)
            ob = ob_tiles[(hp, nch)]
            first = mt == 0
            last = mt == MT - 1
            nc.tensor.matmul(
                ob[0:64, :], lhsT=ones64[:], rhs=as_bf16(e, 0),
                start=first, stop=last, skip_group_check=True,
            )
            nc.tensor.matmul(
                ob[64:128, :], lhsT=ones64[:], rhs=as_bf16(e, 1),
                start=first, stop=last, skip_group_check=True,
            )
            if last:
                # normalize this 512-query chunk: o_norm = oA * recip_fast(oB)
                oa = oa_tiles[(hp, nch)]
                r = rpool.tile([P, 512], dt.float32, name=f"r{hp}_{nch}", tag="r")
                nc.vector.reciprocal_approx_fast(r[:], ob[:])
                nc.vector.tensor_mul(
                    on_all[:, hp * NQ + nch * 512:hp * NQ + (nch + 1) * 512],
                    oa[:], r[:])
                del oa_tiles[(hp, nch)], ob_tiles[(hp, nch)]

        NSS = len(steps) // 2
        emit_qk(0)
        emit_qk(1)
        for ss in range(NSS + 1):
            if ss < NSS:
                for st in (2 * ss, 2 * ss + 1):
                    hp, mt, nch = steps[st]
                    if mt == 0 and (hp, nch) not in oa_tiles:
                        oa_tiles[(hp, nch)] = opool.tile(
                            [P, 512], dt.float32, tag="oa", name=f"oa{hp}_{nch}")
                        ob_tiles[(hp, nch)] = obpool.tile(
                            [P, 512], dt.float32, tag="ob", name=f"ob{hp}_{nch}")
                emit_exp(2 * ss)
                emit_exp(2 * ss + 1)
            if ss > 0:
                emit_pv(2 * (ss - 1))
                emit_pv(2 * (ss - 1) + 1)
                emit_pv_ones(2 * (ss - 1))
                emit_pv_ones(2 * (ss - 1) + 1)
            if 2 * (ss + 1) < len(steps):
                emit_qk(2 * (ss + 1))
            if 2 * (ss + 1) + 1 < len(steps):
                emit_qk(2 * (ss + 1) + 1)

        # ---------------- phase D: proj + residual + out ----------------
        on_r = on_all[:].rearrange("p (kt q) -> p kt q", kt=CT)
        for ct in range(CT):
            # reuse the s-slots (freed as the tail pairs finish) so proj
            # overlaps the last accumulation group
            ys = spool.tile([P, NQ], dt.float32, tag="s", name=f"yps{ct}")
            for nch in range(NQ // 512):
                for pidx in range(2):
                    nc.tensor.matmul(
                        ys[:, nch * 512:(nch + 1) * 512],
                        lhsT=w_pair("p", pidx, ct),
                        rhs=on_r[:, 2 * pidx:2 * pidx + 2,
                                 nch * 512:(nch + 1) * 512],
                        start=(pidx == 0), stop=(pidx == 1), perf_mode=DR,
                    )
            ot = outpool.tile([P, NQ], dt.float32, name=f"ot{ct}", tag="ot")
            # y + (cout + resid), DMA each half as soon as it's done
            nc.vector.tensor_add(ot[:], ys[:], rsd_tiles[ct][:])
            for nch in range(NQ // 512):
                half = slice(nch * 512, (nch + 1) * 512)
                deng = nc.sync if (2 * ct + nch) % 2 == 0 else nc.scalar
                deng.dma_start(y_d.ap()[ct * P:(ct + 1) * P, half], ot[:, half])

    nc.compile()
    return nc


def make_in_maps(inputs):
    x = np.asarray(inputs["x"], dtype=np.float32).reshape(2, C, N)
    gn_w = np.asarray(inputs["gn_w"], np.float32)
    gn_b = np.asarray(inputs["gn_b"], np.float32)
    wq_w = np.asarray(inputs["wq_w"], np.float32)
    wk_w = np.asarray(inputs["wk_w"], np.float32)
    wv_w = np.asarray(inputs["wv_w"], np.float32)
    wp_w = np.asarray(inputs["proj_w"], np.float32)
    wq_b = np.asarray(inputs["wq_b"], np.float32)
    wv_b = np.asarray(inputs["wv_b"], np.float32)
    pj_b = np.asarray(inputs["proj_b"], np.float32)

    def t4(v):
        return np.ascontiguousarray(np.asarray(v, np.float32).reshape(CT, P).T)

    G = 32
    wpT = np.ascontiguousarray(wp_w.T).astype(FP8)
    per_batch = []
    for b in range(2):
        xg = x[b].reshape(G, C // G * N)
        mu = xg.mean(axis=1)
        var = xg.var(axis=1)
        a = gn_w * np.repeat(1.0 / np.sqrt(var + EPS), C // G)
        bb = gn_b - np.repeat(mu, C // G) * a
        wsq = np.ascontiguousarray(a[:, None] * wq_w.T).astype(FP8)
        wsk = np.ascontiguousarray(a[:, None] * wk_w.T).astype(FP8)
        wsv = np.ascontiguousarray(a[:, None] * wv_w.T).astype(FP8)
        qbias = wq_w @ bb + wq_b
        vb = wv_w @ bb + wv_b
        co = wp_w @ vb + pj_b
        per_batch.append(dict(
            wsq=wsq, wsk=wsk, wsv=wsv, wpT=wpT,
            qb4=t4(qbias), cout=co,
        ))

    in_maps = []
    for core in range(8):
        b, r = core // 4, core % 4
        nq0 = r * NQ
        rolled = np.roll(x[b], -nq0, axis=1)
        m = dict(per_batch[b])
        co = m.pop("cout")
        m["xb"] = rolled.astype(FP8)
        # residual + cout pre-added on the host
        m["rsdc"] = np.ascontiguousarray(x[b][:, nq0:nq0 + NQ] + co[:, None]).astype(BF16)
        in_maps.append(m)
    return in_maps


def assemble(results):
    out = np.empty((2, C, N), np.float32)
    for core in range(8):
        b, r = core // 4, core % 4
        out[b][:, r * NQ:(r + 1) * NQ] = results[core]["y"]
    return out.reshape(2, C, 64, 64)


def get_program():
    if "nc" not in _CACHE:
        _CACHE["nc"] = build_program()
    return _CACHE["nc"]


def kernel(**inputs):
    nc = get_program()
    in_maps = make_in_maps(inputs)
    res = run_bass_kernel_spmd(nc, in_maps, core_ids=list(range(8)))
    return assemble(res.results)
